# revision 31
# baseline (speedup 1.0000x reference)
"""Trainium2 Bass kernel for nn_Attention (additive/Bahdanau-style attention).

Math (reference):
    enc [S,B,2H] -> [B,S,2H]
    energy  = tanh(h @ Wh^T + enc @ We^T + b)    # [B,S,H]
    logits  = energy . v                         # [B,S]
    out     = softmax(logits, axis=S)            # [B,S]

Sharding: data-parallel over batch. B=16 rows over 8 NeuronCores -> 2 rows
per core; attn weights replicated. No collectives.

fp8 + exact-repair design (per core):
  - The big matmul e_projT = We^T.T @ encT (K=2048) runs in fp8(e4m3)
    DoubleRow mode: 2 K-tiles per instruction at 0.5 cyc/row -> 2x the fp16
    PE rate (~56us instead of ~112us of PE time).
  - fp8 noise gives logit errors ~0.25 abs, way over the rel-err budget for
    a sharply peaked softmax.  But softmax rows here concentrate on a few
    top entries (mass beyond top-32 < 2e-3), so we repair: per 64-wide
    s-window, DVE max8 selects the top-8 fp8 logits (128 candidates/row,
    a superset of everything that matters), match_replace knocks them down
    to -40 in the base row, dma_gather fetches those 128 enc columns in
    fp16, a small fp16 matmul recomputes their logits exactly, and the
    patched exp values are scattered into the output row over the base
    write (both on the in-order gpsimd dynamic DMA queue).  Host-simulated
    rel err of this exact scheme: 7.3e-4 (fp16 baseline: 6.8e-4).
  - Logit rows are produced replicated across partitions (the v-dot
    partition-sum matmul uses a [128,128] ones lhsT at the same cost as a
    [128,1] one), so the [1,1024] -> [128,64] window reshape for max8 is a
    single revisit-free SBUF DMA, and the dma_gather index tile comes out
    replicated across the 8 gpsimd cores for free.
  - energy tanh is fused on ScalarE: tanh(psum * 2^-16 + (Wh h + b)[o]) --
    the 2^-16 undoes the fp8 quantization scales (enc x16, W x4096).
  - softmax: exp(x - 40) with a constant shift (logits ~[-36, 37]); the
    match-replaced entries contribute exp(-80) ~= 0, repaired entries
    re-enter via the patch term, accum_out gives the denominator.
  - schedule: block (0,0) runs kp-outer so the PE consumes (wet8, enc8)
    DMA pairs as they land during the DMA-bound prefix; later blocks run
    mt-outer (1-2 PSUM banks live) with deferred work -- the previous
    chunk's ones-matmul, the previous row's selection + repair matmuls --
    injected between mt groups so the in-order PE queue never waits on
    DVE/DMA chains.
  - ~2us of junk matmuls pre-warm the PE HAM clock gate during the
    DMA prologue.
"""

from contextlib import ExitStack

import ml_dtypes
import numpy as np

import concourse.bacc as bacc
import concourse.mybir as mybir
import concourse.tile as tile
from concourse import bass_isa
from concourse.bass import IndirectOffsetOnAxis
from concourse.bass_utils import run_bass_kernel_spmd

H = 1024
B = 16
S = 1024
E = 2 * H
NCORES = 8
BL = B // NCORES        # 2 batch rows per core

PT = 128                # partition tile
NT = 512                # free-dim tile (one fp32 PSUM bank)
KP = E // (2 * PT)      # 8 DoubleRow K-pair tiles in the main matmul
KT = E // PT            # 16 fp16 K-tiles (repair matmul)
MT = H // PT            # 8 output-feature tiles
ST = S // NT            # 2 seq chunks
KT_H = H // PT          # 8 K-tiles for h_proj
W = 64                  # selection window width
NSEL = 128              # repaired columns per row (top-8 per window)

SC_E = 16.0             # fp8 quantization scales (center e4m3's range)
SC_W = 4096.0
UNSCALE = 1.0 / (SC_E * SC_W)

F32 = mybir.dt.float32
F32R = mybir.dt.float32r
F16 = mybir.dt.float16
F8 = mybir.dt.float8e4
U16 = mybir.dt.uint16
I16 = mybir.dt.int16
I32 = mybir.dt.int32
AF = mybir.ActivationFunctionType
DR = mybir.MatmulPerfMode.DoubleRow

# cf layout: [128, 16(bt) + 8(vt) + 1(nshift) + 16(ones)]
CF_BT, CF_VT, CF_NS, CF_ONES = 0, KT_H * BL, KT_H * BL + MT, KT_H * BL + MT + 1
CF_N = CF_ONES + 16

REPAIR = True


def build(repair=REPAIR, dbg=False):
    nc = bacc.Bacc("TRN2", target_bir_lowering=False, debug=False)

    enc8 = nc.dram_tensor("enc8", [BL, KP, PT, 2, S], F8, kind="ExternalInput").ap()
    wet8 = nc.dram_tensor("wet8", [KP, PT, 2, H], F8, kind="ExternalInput").ap()
    enc16 = nc.dram_tensor("enc16", [BL, S, E], F16, kind="ExternalInput").ap()
    wet16 = nc.dram_tensor("wet16", [KT, PT, H], F16, kind="ExternalInput").ap()
    wht = nc.dram_tensor("wht", [H, H], F16, kind="ExternalInput").ap()
    ht = nc.dram_tensor("ht", [PT, KT_H * BL], F16, kind="ExternalInput").ap()
    cf = nc.dram_tensor("cf", [PT, CF_N], F32, kind="ExternalInput").ap()
    vrep = nc.dram_tensor("vrep", [PT, MT * PT], F16, kind="ExternalInput").ap()
    pcol = nc.dram_tensor("pcol", [PT, 1], F32, kind="ExternalInput").ap()
    out = nc.dram_tensor("out", [1, BL * S], F32, kind="ExternalOutput").ap()
    hp_dram = nc.dram_tensor("hp_scratch", [BL, H], F32).ap()
    exl_dram = nc.dram_tensor("exl_scratch", [BL, NSEL], F16).ap()
    lg_dram = nc.dram_tensor("lg_scratch", [BL, 8 * S], F32).ap()
    if dbg:
        dbg_hpb = nc.dram_tensor("dbg_hpb", [PT, KT_H * BL], F32, kind="ExternalOutput").ap()
        dbg_lg = nc.dram_tensor("dbg_lg", [PT, S], F32, kind="ExternalOutput").ap()
        dbg_lgr = nc.dram_tensor("dbg_lgr", [PT, W], F32, kind="ExternalOutput").ap()
        dbg_mx = nc.dram_tensor("dbg_mx", [PT, 8], F32, kind="ExternalOutput").ap()
        dbg_gidxf = nc.dram_tensor("dbg_gidxf", [PT, 8], F32, kind="ExternalOutput").ap()
        dbg_exb = nc.dram_tensor("dbg_exb", [PT, W], F32, kind="ExternalOutput").ap()
        dbg_res = nc.dram_tensor("dbg_res", [16, W], F32, kind="ExternalOutput").ap()
        dbg_exl = nc.dram_tensor("dbg_exl", [16, NSEL], F16, kind="ExternalOutput").ap()
        dbg_gso = nc.dram_tensor("dbg_gso", [1, NSEL], F32, kind="ExternalOutput").ap()
        dbg_zz = nc.dram_tensor("dbg_zz", [1, 2], F32, kind="ExternalOutput").ap()
        dbg_G = nc.dram_tensor("dbg_G", [PT, KT * NSEL], F16, kind="ExternalOutput").ap()

    with tile.TileContext(nc) as tc, ExitStack() as ctx:
        constp = ctx.enter_context(tc.tile_pool(name="constp", bufs=1))
        wet8p = ctx.enter_context(tc.tile_pool(name="wet8p", bufs=KP))
        wet16p = ctx.enter_context(tc.tile_pool(name="wet16p", bufs=1))
        whtp = ctx.enter_context(tc.tile_pool(name="whtp", bufs=1))
        encp = ctx.enter_context(tc.tile_pool(name="encp", bufs=2 * KP))
        hpp = ctx.enter_context(tc.tile_pool(name="hpp", bufs=1))
        engp = ctx.enter_context(tc.tile_pool(name="engp", bufs=4))
        accp = ctx.enter_context(tc.tile_pool(name="accp", bufs=3))
        selp = ctx.enter_context(tc.tile_pool(name="selp", bufs=2))
        gp = ctx.enter_context(tc.tile_pool(name="gp", bufs=2))
        psp = ctx.enter_context(tc.tile_pool(name="psp", bufs=8, space="PSUM"))

        # ---- constants (ht first: the very first matmul needs it) -------
        ht_sb = constp.tile([PT, KT_H * BL], F16)
        nc.sync.dma_start(ht_sb[:], ht[:])
        cf_sb = constp.tile([PT, CF_N], F32)
        nc.sync.dma_start(cf_sb[:], cf[:])
        vrep_sb = constp.tile([PT, MT * PT], F16)
        nc.sync.dma_start(vrep_sb[:], vrep[:])
        pcol_sb = constp.tile([PT, 1], F32)
        nc.sync.dma_start(pcol_sb[:], pcol[:])


        bt_sb = cf_sb[:, CF_BT:CF_VT]
        vt_sb = cf_sb[:, CF_VT:CF_NS]
        nshift = cf_sb[:, CF_NS : CF_NS + 1]
        onesf = cf_sb[:, CF_ONES : CF_ONES + 16]

        # phase A weights stream on the gpsimd ring, off the main sync queue
        wht_sb = whtp.tile([PT, KT_H * H], F16, name="wht_sb")
        wht_v = wht_sb[:].rearrange("p (k o) -> p k o", k=KT_H)
        for kt in range(KT_H):
            nc.gpsimd.dma_start(wht_v[:, kt, :], wht[kt * PT : (kt + 1) * PT, :])
        # repair weights: big (4MB) but not needed until the first repair,
        # emitted on gpsimd after wht so the sync queue owns the prefix BW
        wet16_sb = wet16p.tile([PT, KT * H], F16, name="wet16_sb")
        wet16_v = wet16_sb[:].rearrange("p (k o) -> p k o", k=KT)
        for kt in range(KT):
            nc.gpsimd.dma_start(wet16_v[:, kt, :], wet16[kt])

        # HAM pre-warm: junk matmuls while the DMA prologue streams
        junk_ps = psp.tile([1, 2], F32, tag="ps", name="junk_ps2")
        for _ in range(60):
            nc.tensor.matmul(
                junk_ps[:], ht_sb[:, 0:1], ht_sb[:, 0:2],
                start=True, stop=True, skip_group_check=True,
            )

        # ---- phase A: hpb[o-tile][o, b] = (Wh @ h + attn_b) -------------
        php = [
            psp.tile([BL, NT], F32, tag="ps", name=f"php{oc}")
            for oc in range(H // NT)
        ]
        for kt in range(KT_H):
            for oc in range(H // NT):
                nc.tensor.matmul(
                    php[oc][:],
                    ht_sb[:, kt * BL : (kt + 1) * BL],
                    wht_v[:, kt, oc * NT : (oc + 1) * NT],
                    start=(kt == 0),
                    stop=(kt == KT_H - 1),
                )
        hp_sb = hpp.tile([BL, H], F32)
        for oc in range(H // NT):
            nc.scalar.copy(hp_sb[:, oc * NT : (oc + 1) * NT], php[oc][:])
        nc.gpsimd.dma_start(hp_dram[:], hp_sb[:])
        hpt_sb = hpp.tile([PT, KT_H * BL], F32, name="hpt_sb")
        for b in range(BL):
            nc.gpsimd.dma_start(
                hpt_sb[:].rearrange("p (m b) -> p m b", b=BL)[:, :, b],
                hp_dram[b].rearrange("(m p) -> p m", p=PT),
            )
        hpb_sb = hpp.tile([PT, KT_H * BL], F32, name="hpb_sb")
        nc.vector.tensor_add(hpb_sb[:], hpt_sb[:], bt_sb[:])
        if dbg:
            nc.gpsimd.dma_start(dbg_hpb[:], hpb_sb[:])

        # ---- main fp8 blocks -------------------------------------------
        wet8_tiles = [None] * KP

        def load_enc8(b, st):
            ts = []
            for kp in range(KP):
                t = encp.tile([PT, 2, NT], F8, name="enc_t")
                nc.sync.dma_start(
                    t[:], enc8[b, kp, :, :, st * NT : (st + 1) * NT]
                )
                ts.append(t)
            return ts

        def tanh_mt(pe_psum, b, mt):
            # fp16 tanh output feeds the PE v-dot (vrep lhsT) directly
            en = engp.tile([PT, NT], F16, name="en", tag="en")
            nc.scalar.activation(
                en[:], pe_psum[:], AF.Tanh,
                bias=hpb_sb[:, mt * BL + b : mt * BL + b + 1],
                scale=UNSCALE,
            )
            return en

        def dump_lg(pa, b, st):
            # 8 replicated logit rows -> DRAM (selection reloads them in
            # window layout); strided dst: row copies are 1024 apart
            lgs = accp.tile([8, NT], F32, name="lgs", tag="lgs")
            nc.scalar.copy(lgs[:], pa[0:8, :])
            nc.gpsimd.dma_start(
                lg_dram[b]
                .rearrange("(g x) -> g x", g=8)[:, st * NT : (st + 1) * NT],
                lgs[:],
            )

        def vd(pa, en, mt):
            nc.tensor.matmul(
                pa[:], vrep_sb[:, mt * PT : (mt + 1) * PT], en[:],
                start=(mt == 0), stop=(mt == MT - 1),
            )

        def block_kpouter(b, st, with_wet=False):
            pes = [
                psp.tile([PT, NT], F32, tag="ps", name=f"pes_{b}{st}_{mt}")
                for mt in range(MT)
            ]
            pa = psp.tile([PT, NT], F32, tag="ps", name=f"pa{b}{st}")
            ens = [None] * MT
            for kp in range(KP):
                if with_wet:
                    wt = wet8p.tile([PT, 2, H], F8, name="wet8_t")
                    nc.sync.dma_start(wt[:], wet8[kp])
                    wet8_tiles[kp] = wt
                t = encp.tile([PT, 2, NT], F8, name="enc_t")
                nc.sync.dma_start(t[:], enc8[b, kp, :, :, st * NT : (st + 1) * NT])
                for mt in range(MT):
                    nc.tensor.matmul(
                        pes[mt][:],
                        wet8_tiles[kp][:, :, mt * PT : (mt + 1) * PT],
                        t[:],
                        start=(kp == 0),
                        stop=(kp == KP - 1),
                        perf_mode=DR,
                    )
                    if kp == KP - 1:
                        # tanh chases the last kp group mt by mt
                        ens[mt] = tanh_mt(pes[mt], b, mt)
                        if mt >= 1:
                            vd(pa, ens[mt - 1], mt - 1)
            vd(pa, ens[MT - 1], MT - 1)
            dump_lg(pa, b, st)

        def block_mtouter(b, st, etiles, hooks=None):
            pa = psp.tile([PT, NT], F32, tag="ps", name=f"pa{b}{st}")
            prev_en = None
            for mt in range(MT):
                if hooks and mt in hooks:
                    for fn in hooks[mt]:
                        fn()
                pe = psp.tile([PT, NT], F32, tag="ps", name="pe")
                for kp in range(KP):
                    nc.tensor.matmul(
                        pe[:],
                        wet8_tiles[kp][:, :, mt * PT : (mt + 1) * PT],
                        etiles[kp][:],
                        start=(kp == 0),
                        stop=(kp == KP - 1),
                        perf_mode=DR,
                    )
                en = tanh_mt(pe, b, mt)
                if prev_en is not None:
                    vd(pa, prev_en, mt - 1)
                prev_en = en
            vd(pa, prev_en, MT - 1)
            dump_lg(pa, b, st)

        # ---- per-row selection + repair --------------------------------
        row_state = {}

        def select_row(b):
            # windowed logits [128,64]: partition p holds window p%16, 8
            # replicas (one per gpsimd core); single 3-level strided load
            q = nc.sync if b == 1 else nc.gpsimd
            lgr = selp.tile([PT, W], F32, name="lgr", tag=f"lgr{b}")
            for g in range(8):
                q.dma_start(
                    lgr[16 * g : 16 * (g + 1), :],
                    lg_dram[b, g * S : (g + 1) * S].rearrange(
                        "(q f) -> q f", q=16),
                )
            mx = selp.tile([PT, 8], F32, name="mx", tag=f"mx{b}")
            nc.vector.max(mx[:], lgr[:])
            mi = selp.tile([PT, 8], U16, name="mi", tag=f"mi{b}")
            nc.vector.max_index(mi[:], mx[:], lgr[:])
            # row max on every partition -> per-row exp shift (exp args stay
            # in fp16 range regardless of the row's logit scale)
            rmax = selp.tile([PT, 1], F32, name="rmax", tag=f"rmax{b}")
            nc.gpsimd.partition_all_reduce(
                rmax[:], mx[:, 0:1], 128, bass_isa.ReduceOp.max
            )
            nsh = selp.tile([PT, 1], F32, name="nsh", tag=f"nsh{b}")
            nc.vector.tensor_scalar_mul(nsh[:], rmax[:], -1.0)
            gidxf = selp.tile([PT, 8], F32, name="gidxf", tag=f"gidxf{b}")
            nc.vector.tensor_scalar_add(gidxf[:], mi[:], pcol_sb[:, 0:1])
            gidx = selp.tile([PT, 8], I16, name="gidx", tag=f"gidx{b}")
            nc.vector.tensor_copy(gidx[:], gidxf[:])
            if dbg and b == 0:
                nc.gpsimd.dma_start(dbg_lgr[:], lgr[:])
                nc.gpsimd.dma_start(dbg_mx[:], mx[:])
                nc.gpsimd.dma_start(dbg_gidxf[:], gidxf[:])
            if repair:
                lgrep = selp.tile([PT, W], F32, name="lgrep", tag=f"lgrep{b}")
                nc.vector.match_replace(lgrep[:], mx[:], lgr[:], -100.0)
            else:
                lgrep = lgr
            # base exp + per-window partial denominators
            exb = selp.tile([PT, W], F32, name="exb", tag=f"exb{b}")
            zb = selp.tile([PT, 1], F32, name="zb", tag=f"zb{b}")
            nc.scalar.activation(
                exb[:], lgrep[:], AF.Exp, bias=nsh[:, 0:1], accum_out=zb[:]
            )
            G = gp.tile([PT, KT * NSEL], F16, name="G", tag=f"G{b}")
            if repair:
                nc.gpsimd.dma_gather(
                    G[:].rearrange("p (k c) -> p k c", k=KT),
                    enc16[b],
                    gidx[:],
                    NSEL,
                    NSEL,
                    E,
                    transpose=True,
                )
            row_state[b] = dict(G=G, exb=exb, zb=zb, mi=mi, nsh=nsh)

        def repair_matmuls(b):
            st_ = row_state[b]
            Gv = st_["G"][:].rearrange("p (k c) -> p k c", k=KT)
            prs = []
            for mt in range(MT):
                pr = psp.tile([PT, NSEL], F32, tag="ps", name=f"pr{b}{mt}")
                for kt in range(KT):
                    nc.tensor.matmul(
                        pr[:],
                        wet16_v[:, kt, mt * PT : (mt + 1) * PT],
                        Gv[:, kt, :],
                        start=(kt == 0),
                        stop=(kt == KT - 1),
                    )
                prs.append(pr)
            st_["prs"] = prs

        def repair_rest(b):
            st_ = row_state[b]
            exl = None
            if repair:
                accr = accp.tile([PT, NSEL], F32, name="accr", tag="accr")
                for mt in range(MT):
                    enr = engp.tile([PT, NSEL], F32, name="enr", tag="enr")
                    nc.scalar.activation(
                        enr[:], st_["prs"][mt][:], AF.Tanh,
                        bias=hpb_sb[:, mt * BL + b : mt * BL + b + 1],
                    )
                    if mt == 0:
                        nc.vector.tensor_scalar_mul(accr[:], enr[:], vt_sb[:, 0:1])
                    else:
                        nc.vector.scalar_tensor_tensor(
                            accr[:], enr[:], vt_sb[:, mt : mt + 1], accr[:],
                            mybir.AluOpType.mult, mybir.AluOpType.add,
                        )
                # exact logits, replicated over 16 partitions
                lex = psp.tile([16, NSEL], F32, tag="ps", name="lex")
                nc.tensor.matmul(
                    lex[:], onesf[:, 0:16], accr[:], start=True, stop=True)
                exl = selp.tile([16, NSEL], F16, name="exl", tag=f"exl{b}")
                zr = selp.tile([16, 1], F32, name="zr", tag=f"zr{b}")
                nc.scalar.activation(
                    exl[:], lex[:], AF.Exp, bias=st_["nsh"][0:16, 0:1],
                    accum_out=zr[:],
                )
            # denominator: window partials + repaired sum, one PSUM group
            zps = psp.tile([1, 1], F32, tag="ps", name="zps")
            nc.tensor.matmul(
                zps[:], onesf[0:16, 0:1], st_["zb"][0:16, :],
                start=True, stop=not repair,
            )
            if repair:
                nc.tensor.matmul(
                    zps[:], onesf[0:1, 0:1], zr[0:1, 0:1], start=False, stop=True
                )
            rs = selp.tile([1, 1], F32, name="rs", tag=f"rs{b}")
            nc.vector.reciprocal(rs[:], zps[:])
            rzp = psp.tile([16, 1], F32, tag="ps", name="rzp")
            nc.tensor.matmul(rzp[:], onesf[0:1, 0:16], rs[:], start=True, stop=True)
            rz16 = selp.tile([16, 1], F32, name="rz16", tag=f"rz16{b}")
            nc.scalar.copy(rz16[:], rzp[:])
            # normalized base row + patch values
            exbp = st_["exb"][0:16, :]
            if repair:
                # exl row (slot order c = j*16+p) -> [16,8] via DRAM reorder,
                # then a per-partition SBUF scatter into the 64-wide windows
                nc.gpsimd.dma_start(exl_dram[b : b + 1, :], exl[0:1, :])
                exv16 = selp.tile([16, 8], F16, name="exv16", tag=f"exv16{b}")
                nc.gpsimd.dma_start(
                    exv16[:], exl_dram[b].rearrange("(j p) -> p j", p=16)
                )
                pg = selp.tile([16, W], F16, name="pg", tag=f"pg{b}")
                nc.gpsimd.local_scatter(
                    pg[:], exv16[:], st_["mi"][0:16, :].bitcast(I16), 16, W, 8
                )
                exbsum = selp.tile([16, W], F32, name="exbsum", tag=f"exbs{b}")
                nc.vector.tensor_add(exbsum[:], exbp, pg[:])
                exbp = exbsum[:]
            res = selp.tile([16, W], F32, name="res", tag=f"res{b}")
            nc.vector.tensor_scalar_mul(res[:], exbp, rz16[:, 0:1])
            if dbg and b == 0:
                nc.gpsimd.dma_start(dbg_res[:], res[:])
                if repair:
                    nc.gpsimd.dma_start(dbg_exl[:], exl[:])
                    nc.gpsimd.dma_start(dbg_G[:], st_["G"][:])
                zz = selp.tile([1, 2], F32, name="zz", tag="zz")
                nc.vector.tensor_copy(zz[:, 0:1], zps[:])
                nc.vector.tensor_copy(zz[:, 1:2], rs[:])
                nc.gpsimd.dma_start(dbg_zz[:], zz[:])
            nc.sync.dma_start(
                out[0, b * S : (b + 1) * S].rearrange("(q f) -> q f", q=16),
                res[:],
            )

        # ---- schedule ---------------------------------------------------
        block_kpouter(0, 0, with_wet=True)
        et01 = load_enc8(0, 1)
        block_mtouter(0, 1, et01)
        et10 = load_enc8(1, 0)
        block_mtouter(
            1, 0, et10,
            hooks={0: [lambda: select_row(0)]},
        )
        et11 = load_enc8(1, 1)
        hooks11 = {}
        if repair:
            hooks11[1] = [lambda: repair_matmuls(0)]
            hooks11[5] = [lambda: repair_rest(0)]
        else:
            hooks11[1] = [lambda: repair_rest(0)]
        block_mtouter(1, 1, et11, hooks=hooks11)
        select_row(1)
        if repair:
            repair_matmuls(1)
        repair_rest(1)

    nc.compile()
    return nc


_NC_CACHE = {}


def _get_nc(repair=REPAIR, dbg=False):
    key = (repair, dbg)
    if key not in _NC_CACHE:
        _NC_CACHE[key] = build(repair, dbg)
    return _NC_CACHE[key]


def make_in_maps(hidden_state, encoder_outputs, attn_w, attn_b, v):
    hidden_state = np.asarray(hidden_state, dtype=np.float32)
    encoder_outputs = np.asarray(encoder_outputs, dtype=np.float32)
    attn_w = np.asarray(attn_w, dtype=np.float32)
    attn_b = np.asarray(attn_b, dtype=np.float32)
    v = np.asarray(v, dtype=np.float32)

    w_e_t = np.ascontiguousarray(attn_w[:, H:].T)          # [E, H]
    wht_t = np.ascontiguousarray(attn_w[:, :H].T).astype(np.float16)
    encb = encoder_outputs.transpose(1, 0, 2)              # [B, S, E]

    # fp8 operands (scaled into e4m3's sweet spot)
    wet8_t = np.asarray(w_e_t * SC_W, dtype=ml_dtypes.float8_e4m3)
    wet8_t = np.ascontiguousarray(
        wet8_t.reshape(KP, 2, PT, H).transpose(0, 2, 1, 3))   # [KP,128,2,H]
    wet16_t = np.ascontiguousarray(w_e_t.reshape(KT, PT, H)).astype(np.float16)

    bt_t = np.repeat(
        attn_b.reshape(MT, PT).T[:, :, None], BL, axis=2
    ).reshape(PT, MT * BL)
    vt_t = np.ascontiguousarray(v.reshape(MT, PT).T)
    cf_t = np.concatenate(
        [bt_t, vt_t, np.full((PT, 1), -40.0, np.float32),
         np.ones((PT, 16), np.float32)], axis=1,
    ).astype(np.float32)
    pcol_t = ((np.arange(PT) % 16) * W).astype(np.float32).reshape(PT, 1)
    vrep_t = np.ascontiguousarray(
        np.repeat(v.reshape(MT, PT, 1), PT, axis=2).transpose(1, 0, 2)
        .reshape(PT, MT * PT)).astype(np.float16)

    in_maps = []
    for i in range(NCORES):
        rows = slice(i * BL, (i + 1) * BL)
        enc_c = encb[rows]                                 # [BL, S, E]
        encT = enc_c.transpose(0, 2, 1)                    # [BL, E, S]
        enc8_t = np.asarray(encT * SC_E, dtype=ml_dtypes.float8_e4m3)
        enc8_t = np.ascontiguousarray(
            enc8_t.reshape(BL, KP, 2, PT, S).transpose(0, 1, 3, 2, 4))
        in_maps.append(
            {
                "enc8": enc8_t,
                "wet8": wet8_t,
                "enc16": np.ascontiguousarray(enc_c).astype(np.float16),
                "wet16": wet16_t,
                "wht": wht_t,
                "ht": np.ascontiguousarray(
                    hidden_state[rows].T.reshape(KT_H, PT, BL)
                    .transpose(1, 0, 2).reshape(PT, KT_H * BL)
                ).astype(np.float16),
                "cf": cf_t,
                "vrep": vrep_t,
                "pcol": pcol_t,
            }
        )
    return in_maps


def run(inputs, trace=False, compute_dtype=None, dbg=False, **spmd_kwargs):
    nc = _get_nc(dbg=dbg)
    in_maps = make_in_maps(**inputs)
    res = run_bass_kernel_spmd(
        nc, in_maps, core_ids=list(range(NCORES)), trace=trace, **spmd_kwargs
    )
    out = np.concatenate(
        [res.results[i]["out"].reshape(BL, S) for i in range(NCORES)], axis=0
    )
    return out.astype(np.float32), res


def kernel(**inputs):
    out, _ = run(inputs, trace=False)
    return out


# revision 32
# speedup vs baseline: 1.0135x; 1.0135x over previous
"""Trainium2 Bass kernel for nn_Attention (additive/Bahdanau-style attention).

Math (reference):
    enc [S,B,2H] -> [B,S,2H]
    energy  = tanh(h @ Wh^T + enc @ We^T + b)    # [B,S,H]
    logits  = energy . v                         # [B,S]
    out     = softmax(logits, axis=S)            # [B,S]

Sharding: data-parallel over batch. B=16 rows over 8 NeuronCores -> 2 rows
per core; attn weights replicated. No collectives.

fp8 + exact-repair design (per core):
  - The big matmul e_projT = We^T.T @ encT (K=2048) runs in fp8(e4m3)
    DoubleRow mode: 2 K-tiles per instruction at 0.5 cyc/row -> 2x the fp16
    PE rate (~56us instead of ~112us of PE time).
  - fp8 noise gives logit errors ~0.25 abs, way over the rel-err budget for
    a sharply peaked softmax.  But softmax rows here concentrate on a few
    top entries (mass beyond top-32 < 2e-3), so we repair: per 64-wide
    s-window, DVE max8 selects the top-8 fp8 logits (128 candidates/row,
    a superset of everything that matters), match_replace knocks them down
    to -40 in the base row, dma_gather fetches those 128 enc columns in
    fp16, a small fp16 matmul recomputes their logits exactly, and the
    patched exp values are scattered into the output row over the base
    write (both on the in-order gpsimd dynamic DMA queue).  Host-simulated
    rel err of this exact scheme: 7.3e-4 (fp16 baseline: 6.8e-4).
  - Logit rows are produced replicated across partitions (the v-dot
    partition-sum matmul uses a [128,128] ones lhsT at the same cost as a
    [128,1] one), so the [1,1024] -> [128,64] window reshape for max8 is a
    single revisit-free SBUF DMA, and the dma_gather index tile comes out
    replicated across the 8 gpsimd cores for free.
  - energy tanh is fused on ScalarE: tanh(psum * 2^-16 + (Wh h + b)[o]) --
    the 2^-16 undoes the fp8 quantization scales (enc x16, W x4096).
  - softmax: exp(x - 40) with a constant shift (logits ~[-36, 37]); the
    match-replaced entries contribute exp(-80) ~= 0, repaired entries
    re-enter via the patch term, accum_out gives the denominator.
  - schedule: block (0,0) runs kp-outer so the PE consumes (wet8, enc8)
    DMA pairs as they land during the DMA-bound prefix; later blocks run
    mt-outer (1-2 PSUM banks live) with deferred work -- the previous
    chunk's ones-matmul, the previous row's selection + repair matmuls --
    injected between mt groups so the in-order PE queue never waits on
    DVE/DMA chains.
  - ~2us of junk matmuls pre-warm the PE HAM clock gate during the
    DMA prologue.
"""

from contextlib import ExitStack

import ml_dtypes
import numpy as np

import concourse.bacc as bacc
import concourse.mybir as mybir
import concourse.tile as tile
from concourse import bass_isa
from concourse.bass import IndirectOffsetOnAxis
from concourse.bass_utils import run_bass_kernel_spmd

H = 1024
B = 16
S = 1024
E = 2 * H
NCORES = 8
BL = B // NCORES        # 2 batch rows per core

PT = 128                # partition tile
NT = 512                # free-dim tile (one fp32 PSUM bank)
KP = E // (2 * PT)      # 8 DoubleRow K-pair tiles in the main matmul
KT = E // PT            # 16 fp16 K-tiles (repair matmul)
MT = H // PT            # 8 output-feature tiles
ST = S // NT            # 2 seq chunks
KT_H = H // PT          # 8 K-tiles for h_proj
W = 64                  # selection window width
NSEL = 128              # repaired columns per row (top-8 per window)

SC_E = 16.0             # fp8 quantization scales (center e4m3's range)
SC_W = 4096.0
UNSCALE = 1.0 / (SC_E * SC_W)

F32 = mybir.dt.float32
F32R = mybir.dt.float32r
F16 = mybir.dt.float16
F8 = mybir.dt.float8e4
U16 = mybir.dt.uint16
I16 = mybir.dt.int16
I32 = mybir.dt.int32
AF = mybir.ActivationFunctionType
DR = mybir.MatmulPerfMode.DoubleRow

# cf layout: [128, 16(bt) + 8(vt) + 1(nshift) + 16(ones) + 128(prep)]
CF_BT, CF_VT, CF_NS, CF_ONES = 0, KT_H * BL, KT_H * BL + MT, KT_H * BL + MT + 1
CF_PREP = CF_ONES + 16
CF_N = CF_PREP + 128

REPAIR = True


def build(repair=REPAIR, dbg=False):
    nc = bacc.Bacc("TRN2", target_bir_lowering=False, debug=False)

    enc8 = nc.dram_tensor("enc8", [BL, KP, PT, 2, S], F8, kind="ExternalInput").ap()
    wet8 = nc.dram_tensor("wet8", [KP, PT, 2, H], F8, kind="ExternalInput").ap()
    enc16 = nc.dram_tensor("enc16", [BL, S, E], F16, kind="ExternalInput").ap()
    wet16 = nc.dram_tensor("wet16", [KT, PT, H], F16, kind="ExternalInput").ap()
    wht = nc.dram_tensor("wht", [H, H], F16, kind="ExternalInput").ap()
    ht = nc.dram_tensor("ht", [PT, KT_H * BL], F16, kind="ExternalInput").ap()
    cf = nc.dram_tensor("cf", [PT, CF_N], F32, kind="ExternalInput").ap()
    vrep = nc.dram_tensor("vrep", [PT, MT * PT], F16, kind="ExternalInput").ap()
    pcol = nc.dram_tensor("pcol", [PT, 1], F32, kind="ExternalInput").ap()
    out = nc.dram_tensor("out", [1, BL * S], F32, kind="ExternalOutput").ap()
    hp_dram = nc.dram_tensor("hp_scratch", [BL, H], F32).ap()
    exl_dram = nc.dram_tensor("exl_scratch", [BL, NSEL], F16).ap()
    lg_dram = nc.dram_tensor("lg_scratch", [BL, S], F32).ap()
    if dbg:
        dbg_hpb = nc.dram_tensor("dbg_hpb", [PT, KT_H * BL], F32, kind="ExternalOutput").ap()
        dbg_lg = nc.dram_tensor("dbg_lg", [PT, S], F32, kind="ExternalOutput").ap()
        dbg_lgr = nc.dram_tensor("dbg_lgr", [PT, W], F32, kind="ExternalOutput").ap()
        dbg_mx = nc.dram_tensor("dbg_mx", [PT, 8], F32, kind="ExternalOutput").ap()
        dbg_gidxf = nc.dram_tensor("dbg_gidxf", [PT, 8], F32, kind="ExternalOutput").ap()
        dbg_exb = nc.dram_tensor("dbg_exb", [PT, W], F32, kind="ExternalOutput").ap()
        dbg_res = nc.dram_tensor("dbg_res", [16, W], F32, kind="ExternalOutput").ap()
        dbg_exl = nc.dram_tensor("dbg_exl", [16, NSEL], F16, kind="ExternalOutput").ap()
        dbg_gso = nc.dram_tensor("dbg_gso", [1, NSEL], F32, kind="ExternalOutput").ap()
        dbg_zz = nc.dram_tensor("dbg_zz", [1, 2], F32, kind="ExternalOutput").ap()
        dbg_G = nc.dram_tensor("dbg_G", [PT, KT * NSEL], F16, kind="ExternalOutput").ap()

    with tile.TileContext(nc) as tc, ExitStack() as ctx:
        constp = ctx.enter_context(tc.tile_pool(name="constp", bufs=1))
        wet8p = ctx.enter_context(tc.tile_pool(name="wet8p", bufs=KP))
        wet16p = ctx.enter_context(tc.tile_pool(name="wet16p", bufs=1))
        whtp = ctx.enter_context(tc.tile_pool(name="whtp", bufs=1))
        encp = ctx.enter_context(tc.tile_pool(name="encp", bufs=2 * KP))
        hpp = ctx.enter_context(tc.tile_pool(name="hpp", bufs=1))
        engp = ctx.enter_context(tc.tile_pool(name="engp", bufs=4))
        accp = ctx.enter_context(tc.tile_pool(name="accp", bufs=3))
        selp = ctx.enter_context(tc.tile_pool(name="selp", bufs=2))
        gp = ctx.enter_context(tc.tile_pool(name="gp", bufs=2))
        psp = ctx.enter_context(tc.tile_pool(name="psp", bufs=8, space="PSUM"))

        # ---- constants (ht first: the very first matmul needs it) -------
        ht_sb = constp.tile([PT, KT_H * BL], F16)
        nc.sync.dma_start(ht_sb[:], ht[:])
        cf_sb = constp.tile([PT, CF_N], F32)
        nc.sync.dma_start(cf_sb[:], cf[:])
        vrep_sb = constp.tile([PT, MT * PT], F16)
        nc.sync.dma_start(vrep_sb[:], vrep[:])
        pcol_sb = constp.tile([PT, 1], F32)
        nc.sync.dma_start(pcol_sb[:], pcol[:])


        bt_sb = cf_sb[:, CF_BT:CF_VT]
        vt_sb = cf_sb[:, CF_VT:CF_NS]
        nshift = cf_sb[:, CF_NS : CF_NS + 1]
        onesf = cf_sb[:, CF_ONES : CF_ONES + 16]
        prep = cf_sb[0:16, CF_PREP : CF_PREP + 128]

        # phase A weights stream on the gpsimd ring, off the main sync queue
        wht_sb = whtp.tile([PT, KT_H * H], F16, name="wht_sb")
        wht_v = wht_sb[:].rearrange("p (k o) -> p k o", k=KT_H)
        for kt in range(KT_H):
            nc.gpsimd.dma_start(wht_v[:, kt, :], wht[kt * PT : (kt + 1) * PT, :])
        # repair weights: big (4MB) but not needed until the first repair,
        # emitted on gpsimd after wht so the sync queue owns the prefix BW
        wet16_sb = wet16p.tile([PT, KT * H], F16, name="wet16_sb")
        wet16_v = wet16_sb[:].rearrange("p (k o) -> p k o", k=KT)
        for kt in range(KT):
            nc.gpsimd.dma_start(wet16_v[:, kt, :], wet16[kt])

        # HAM pre-warm: junk matmuls while the DMA prologue streams
        junk_ps = psp.tile([1, 2], F32, tag="ps", name="junk_ps2")
        for _ in range(60):
            nc.tensor.matmul(
                junk_ps[:], ht_sb[:, 0:1], ht_sb[:, 0:2],
                start=True, stop=True, skip_group_check=True,
            )

        # ---- phase A: hpb[o-tile][o, b] = (Wh @ h + attn_b) -------------
        php = [
            psp.tile([BL, NT], F32, tag="ps", name=f"php{oc}")
            for oc in range(H // NT)
        ]
        for kt in range(KT_H):
            for oc in range(H // NT):
                nc.tensor.matmul(
                    php[oc][:],
                    ht_sb[:, kt * BL : (kt + 1) * BL],
                    wht_v[:, kt, oc * NT : (oc + 1) * NT],
                    start=(kt == 0),
                    stop=(kt == KT_H - 1),
                )
        hp_sb = hpp.tile([BL, H], F32)
        for oc in range(H // NT):
            nc.scalar.copy(hp_sb[:, oc * NT : (oc + 1) * NT], php[oc][:])
        nc.gpsimd.dma_start(hp_dram[:], hp_sb[:])
        hpt_sb = hpp.tile([PT, KT_H * BL], F32, name="hpt_sb")
        for b in range(BL):
            nc.gpsimd.dma_start(
                hpt_sb[:].rearrange("p (m b) -> p m b", b=BL)[:, :, b],
                hp_dram[b].rearrange("(m p) -> p m", p=PT),
            )
        hpb_sb = hpp.tile([PT, KT_H * BL], F32, name="hpb_sb")
        nc.vector.tensor_add(hpb_sb[:], hpt_sb[:], bt_sb[:])
        if dbg:
            nc.gpsimd.dma_start(dbg_hpb[:], hpb_sb[:])

        # ---- main fp8 blocks -------------------------------------------
        wet8_tiles = [None] * KP

        def load_enc8(b, st):
            ts = []
            for kp in range(KP):
                t = encp.tile([PT, 2, NT], F8, name="enc_t")
                nc.sync.dma_start(
                    t[:], enc8[b, kp, :, :, st * NT : (st + 1) * NT]
                )
                ts.append(t)
            return ts

        def tanh_mt(pe_psum, b, mt):
            # fp16 tanh output feeds the PE v-dot (vrep lhsT) directly
            en = engp.tile([PT, NT], F16, name="en", tag="en")
            nc.scalar.activation(
                en[:], pe_psum[:], AF.Tanh,
                bias=hpb_sb[:, mt * BL + b : mt * BL + b + 1],
                scale=UNSCALE,
            )
            return en

        def dump_lg(pa, b, st):
            # 8 replicated logit rows -> DRAM (selection reloads them in
            # window layout); strided dst: row copies are 1024 apart
            lgs = accp.tile([1, NT], F32, name="lgs", tag="lgs")
            nc.scalar.copy(lgs[:], pa[0:1, :])
            nc.gpsimd.dma_start(
                lg_dram[b : b + 1, st * NT : (st + 1) * NT], lgs[:]
            )

        def vd(pa, en, mt):
            nc.tensor.matmul(
                pa[:], vrep_sb[:, mt * PT : (mt + 1) * PT], en[:],
                start=(mt == 0), stop=(mt == MT - 1),
            )

        def block_kpouter(b, st, with_wet=False):
            pes = [
                psp.tile([PT, NT], F32, tag="ps", name=f"pes_{b}{st}_{mt}")
                for mt in range(MT)
            ]
            pa = psp.tile([PT, NT], F32, tag="ps", name=f"pa{b}{st}")
            ens = [None] * MT
            for kp in range(KP):
                if with_wet:
                    wt = wet8p.tile([PT, 2, H], F8, name="wet8_t")
                    nc.sync.dma_start(wt[:], wet8[kp])
                    wet8_tiles[kp] = wt
                t = encp.tile([PT, 2, NT], F8, name="enc_t")
                nc.sync.dma_start(t[:], enc8[b, kp, :, :, st * NT : (st + 1) * NT])
                for mt in range(MT):
                    nc.tensor.matmul(
                        pes[mt][:],
                        wet8_tiles[kp][:, :, mt * PT : (mt + 1) * PT],
                        t[:],
                        start=(kp == 0),
                        stop=(kp == KP - 1),
                        perf_mode=DR,
                    )
                    if kp == KP - 1:
                        # tanh chases the last kp group mt by mt
                        ens[mt] = tanh_mt(pes[mt], b, mt)
                        if mt >= 1:
                            vd(pa, ens[mt - 1], mt - 1)
            vd(pa, ens[MT - 1], MT - 1)
            dump_lg(pa, b, st)

        def block_mtouter(b, st, etiles, hooks=None):
            pa = psp.tile([PT, NT], F32, tag="ps", name=f"pa{b}{st}")
            prev_en = None
            for mt in range(MT):
                if hooks and mt in hooks:
                    for fn in hooks[mt]:
                        fn()
                pe = psp.tile([PT, NT], F32, tag="ps", name="pe")
                for kp in range(KP):
                    nc.tensor.matmul(
                        pe[:],
                        wet8_tiles[kp][:, :, mt * PT : (mt + 1) * PT],
                        etiles[kp][:],
                        start=(kp == 0),
                        stop=(kp == KP - 1),
                        perf_mode=DR,
                    )
                en = tanh_mt(pe, b, mt)
                if prev_en is not None:
                    vd(pa, prev_en, mt - 1)
                prev_en = en
            vd(pa, prev_en, MT - 1)
            dump_lg(pa, b, st)

        # ---- per-row selection + repair --------------------------------
        row_state = {}

        def select_row(b):
            # one small DMA: [1,1024] logits -> [16,64] windows; selection
            # runs on 16 partitions, the gather index tile is replicated to
            # 128 partitions by a tiny PE matmul against an indicator matrix
            q = nc.sync if b == 1 else nc.gpsimd
            lgr = selp.tile([16, W], F32, name="lgr", tag=f"lgr{b}")
            q.dma_start(
                lgr[:], lg_dram[b].rearrange("(q f) -> q f", q=16)
            )
            mx = selp.tile([16, 8], F32, name="mx", tag=f"mx{b}")
            nc.vector.max(mx[:], lgr[:])
            mi = selp.tile([16, 8], U16, name="mi", tag=f"mi{b}")
            nc.vector.max_index(mi[:], mx[:], lgr[:])
            # row max on every partition -> per-row exp shift (exp args stay
            # in fp16 range regardless of the row's logit scale)
            rmax = selp.tile([16, 1], F32, name="rmax", tag=f"rmax{b}")
            nc.gpsimd.partition_all_reduce(
                rmax[:], mx[:, 0:1], 16, bass_isa.ReduceOp.max
            )
            nsh = selp.tile([16, 1], F32, name="nsh", tag=f"nsh{b}")
            nc.vector.tensor_scalar_mul(nsh[:], rmax[:], -1.0)
            gidxf = selp.tile([16, 8], F32, name="gidxf", tag=f"gidxf{b}")
            nc.vector.tensor_scalar_add(gidxf[:], mi[:], pcol_sb[0:16, 0:1])
            gps = psp.tile([PT, 8], F32, tag="ps", name="gps")
            nc.tensor.matmul(gps[:], prep, gidxf[:], start=True, stop=True)
            gidx = selp.tile([PT, 8], I16, name="gidx", tag=f"gidx{b}")
            nc.vector.tensor_copy(gidx[:], gps[:])
            if dbg and b == 0:
                nc.gpsimd.dma_start(dbg_lgr[0:16, :], lgr[:])
                nc.gpsimd.dma_start(dbg_mx[0:16, :], mx[:])
                nc.gpsimd.dma_start(dbg_gidxf[0:16, :], gidxf[:])
            if repair:
                lgrep = selp.tile([16, W], F32, name="lgrep", tag=f"lgrep{b}")
                nc.vector.match_replace(lgrep[:], mx[:], lgr[:], -100.0)
            else:
                lgrep = lgr
            # base exp + per-window partial denominators
            exb = selp.tile([16, W], F32, name="exb", tag=f"exb{b}")
            zb = selp.tile([16, 1], F32, name="zb", tag=f"zb{b}")
            nc.scalar.activation(
                exb[:], lgrep[:], AF.Exp, bias=nsh[:, 0:1], accum_out=zb[:]
            )
            G = gp.tile([PT, KT * NSEL], F16, name="G", tag=f"G{b}")
            if repair:
                nc.gpsimd.dma_gather(
                    G[:].rearrange("p (k c) -> p k c", k=KT),
                    enc16[b],
                    gidx[:],
                    NSEL,
                    NSEL,
                    E,
                    transpose=True,
                )
            row_state[b] = dict(G=G, exb=exb, zb=zb, mi=mi, nsh=nsh)

        def repair_matmuls(b):
            st_ = row_state[b]
            Gv = st_["G"][:].rearrange("p (k c) -> p k c", k=KT)
            prs = []
            for mt in range(MT):
                pr = psp.tile([PT, NSEL], F32, tag="ps", name=f"pr{b}{mt}")
                for kt in range(KT):
                    nc.tensor.matmul(
                        pr[:],
                        wet16_v[:, kt, mt * PT : (mt + 1) * PT],
                        Gv[:, kt, :],
                        start=(kt == 0),
                        stop=(kt == KT - 1),
                    )
                prs.append(pr)
            st_["prs"] = prs

        def repair_rest(b):
            st_ = row_state[b]
            exl = None
            if repair:
                accr = accp.tile([PT, NSEL], F32, name="accr", tag="accr")
                for mt in range(MT):
                    enr = engp.tile([PT, NSEL], F32, name="enr", tag="enr")
                    nc.scalar.activation(
                        enr[:], st_["prs"][mt][:], AF.Tanh,
                        bias=hpb_sb[:, mt * BL + b : mt * BL + b + 1],
                    )
                    if mt == 0:
                        nc.vector.tensor_scalar_mul(accr[:], enr[:], vt_sb[:, 0:1])
                    else:
                        nc.vector.scalar_tensor_tensor(
                            accr[:], enr[:], vt_sb[:, mt : mt + 1], accr[:],
                            mybir.AluOpType.mult, mybir.AluOpType.add,
                        )
                # exact logits, replicated over 16 partitions
                lex = psp.tile([16, NSEL], F32, tag="ps", name="lex")
                nc.tensor.matmul(
                    lex[:], onesf[:, 0:16], accr[:], start=True, stop=True)
                exl = selp.tile([16, NSEL], F16, name="exl", tag=f"exl{b}")
                zr = selp.tile([16, 1], F32, name="zr", tag=f"zr{b}")
                nc.scalar.activation(
                    exl[:], lex[:], AF.Exp, bias=st_["nsh"][:, 0:1],
                    accum_out=zr[:],
                )
            # denominator: window partials + repaired sum, one PSUM group
            zps = psp.tile([1, 1], F32, tag="ps", name="zps")
            nc.tensor.matmul(
                zps[:], onesf[0:16, 0:1], st_["zb"][:],
                start=True, stop=not repair,
            )
            if repair:
                nc.tensor.matmul(
                    zps[:], onesf[0:1, 0:1], zr[0:1, 0:1], start=False, stop=True
                )
            rs = selp.tile([1, 1], F32, name="rs", tag=f"rs{b}")
            nc.vector.reciprocal(rs[:], zps[:])
            rzp = psp.tile([16, 1], F32, tag="ps", name="rzp")
            nc.tensor.matmul(rzp[:], onesf[0:1, 0:16], rs[:], start=True, stop=True)
            rz16 = selp.tile([16, 1], F32, name="rz16", tag=f"rz16{b}")
            nc.scalar.copy(rz16[:], rzp[:])
            # normalized base row + patch values
            exbp = st_["exb"][:]
            if repair:
                # exl row (slot order c = j*16+p) -> [16,8] via DRAM reorder,
                # then a per-partition SBUF scatter into the 64-wide windows
                nc.gpsimd.dma_start(exl_dram[b : b + 1, :], exl[0:1, :])
                exv16 = selp.tile([16, 8], F16, name="exv16", tag=f"exv16{b}")
                nc.gpsimd.dma_start(
                    exv16[:], exl_dram[b].rearrange("(j p) -> p j", p=16)
                )
                pg = selp.tile([16, W], F16, name="pg", tag=f"pg{b}")
                nc.gpsimd.local_scatter(
                    pg[:], exv16[:], st_["mi"][:].bitcast(I16), 16, W, 8
                )
                exbsum = selp.tile([16, W], F32, name="exbsum", tag=f"exbs{b}")
                nc.vector.tensor_add(exbsum[:], exbp, pg[:])
                exbp = exbsum[:]
            res = selp.tile([16, W], F32, name="res", tag=f"res{b}")
            nc.vector.tensor_scalar_mul(res[:], exbp, rz16[:, 0:1])
            if dbg and b == 0:
                nc.gpsimd.dma_start(dbg_res[:], res[:])
                if repair:
                    nc.gpsimd.dma_start(dbg_exl[:], exl[:])
                    nc.gpsimd.dma_start(dbg_G[:], st_["G"][:])
                zz = selp.tile([1, 2], F32, name="zz", tag="zz")
                nc.vector.tensor_copy(zz[:, 0:1], zps[:])
                nc.vector.tensor_copy(zz[:, 1:2], rs[:])
                nc.gpsimd.dma_start(dbg_zz[:], zz[:])
            nc.sync.dma_start(
                out[0, b * S : (b + 1) * S].rearrange("(q f) -> q f", q=16),
                res[:],
            )

        # ---- schedule ---------------------------------------------------
        block_kpouter(0, 0, with_wet=True)
        et01 = load_enc8(0, 1)
        block_mtouter(0, 1, et01)
        et10 = load_enc8(1, 0)
        block_mtouter(
            1, 0, et10,
            hooks={0: [lambda: select_row(0)]},
        )
        et11 = load_enc8(1, 1)
        hooks11 = {}
        if repair:
            hooks11[1] = [lambda: repair_matmuls(0)]
            hooks11[5] = [lambda: repair_rest(0)]
        else:
            hooks11[1] = [lambda: repair_rest(0)]
        block_mtouter(1, 1, et11, hooks=hooks11)
        select_row(1)
        if repair:
            repair_matmuls(1)
        repair_rest(1)

    nc.compile()
    return nc


_NC_CACHE = {}


def _get_nc(repair=REPAIR, dbg=False):
    key = (repair, dbg)
    if key not in _NC_CACHE:
        _NC_CACHE[key] = build(repair, dbg)
    return _NC_CACHE[key]


def make_in_maps(hidden_state, encoder_outputs, attn_w, attn_b, v):
    hidden_state = np.asarray(hidden_state, dtype=np.float32)
    encoder_outputs = np.asarray(encoder_outputs, dtype=np.float32)
    attn_w = np.asarray(attn_w, dtype=np.float32)
    attn_b = np.asarray(attn_b, dtype=np.float32)
    v = np.asarray(v, dtype=np.float32)

    w_e_t = np.ascontiguousarray(attn_w[:, H:].T)          # [E, H]
    wht_t = np.ascontiguousarray(attn_w[:, :H].T).astype(np.float16)
    encb = encoder_outputs.transpose(1, 0, 2)              # [B, S, E]

    # fp8 operands (scaled into e4m3's sweet spot)
    wet8_t = np.asarray(w_e_t * SC_W, dtype=ml_dtypes.float8_e4m3)
    wet8_t = np.ascontiguousarray(
        wet8_t.reshape(KP, 2, PT, H).transpose(0, 2, 1, 3))   # [KP,128,2,H]
    wet16_t = np.ascontiguousarray(w_e_t.reshape(KT, PT, H)).astype(np.float16)

    bt_t = np.repeat(
        attn_b.reshape(MT, PT).T[:, :, None], BL, axis=2
    ).reshape(PT, MT * BL)
    vt_t = np.ascontiguousarray(v.reshape(MT, PT).T)
    prep_t = (np.arange(128)[None, :] % 16 ==
              np.arange(PT)[:, None]).astype(np.float32)
    cf_t = np.concatenate(
        [bt_t, vt_t, np.full((PT, 1), -40.0, np.float32),
         np.ones((PT, 16), np.float32), prep_t], axis=1,
    ).astype(np.float32)
    pcol_t = ((np.arange(PT) % 16) * W).astype(np.float32).reshape(PT, 1)
    vrep_t = np.ascontiguousarray(
        np.repeat(v.reshape(MT, PT, 1), PT, axis=2).transpose(1, 0, 2)
        .reshape(PT, MT * PT)).astype(np.float16)

    in_maps = []
    for i in range(NCORES):
        rows = slice(i * BL, (i + 1) * BL)
        enc_c = encb[rows]                                 # [BL, S, E]
        encT = enc_c.transpose(0, 2, 1)                    # [BL, E, S]
        enc8_t = np.asarray(encT * SC_E, dtype=ml_dtypes.float8_e4m3)
        enc8_t = np.ascontiguousarray(
            enc8_t.reshape(BL, KP, 2, PT, S).transpose(0, 1, 3, 2, 4))
        in_maps.append(
            {
                "enc8": enc8_t,
                "wet8": wet8_t,
                "enc16": np.ascontiguousarray(enc_c).astype(np.float16),
                "wet16": wet16_t,
                "wht": wht_t,
                "ht": np.ascontiguousarray(
                    hidden_state[rows].T.reshape(KT_H, PT, BL)
                    .transpose(1, 0, 2).reshape(PT, KT_H * BL)
                ).astype(np.float16),
                "cf": cf_t,
                "vrep": vrep_t,
                "pcol": pcol_t,
            }
        )
    return in_maps


def run(inputs, trace=False, compute_dtype=None, dbg=False, **spmd_kwargs):
    nc = _get_nc(dbg=dbg)
    in_maps = make_in_maps(**inputs)
    res = run_bass_kernel_spmd(
        nc, in_maps, core_ids=list(range(NCORES)), trace=trace, **spmd_kwargs
    )
    out = np.concatenate(
        [res.results[i]["out"].reshape(BL, S) for i in range(NCORES)], axis=0
    )
    return out.astype(np.float32), res


def kernel(**inputs):
    out, _ = run(inputs, trace=False)
    return out


# revision 33
# speedup vs baseline: 1.1126x; 1.0977x over previous
"""Trainium2 Bass kernel for nn_Attention (additive/Bahdanau-style attention).

Math (reference):
    enc [S,B,2H] -> [B,S,2H]
    energy  = tanh(h @ Wh^T + enc @ We^T + b)    # [B,S,H]
    logits  = energy . v                         # [B,S]
    out     = softmax(logits, axis=S)            # [B,S]

Sharding: data-parallel over batch. B=16 rows over 8 NeuronCores -> 2 rows
per core; attn weights replicated. No collectives.

fp8 + exact-repair design (per core):
  - The big matmul e_projT = We^T.T @ encT (K=2048) runs in fp8(e4m3)
    DoubleRow mode: 2 K-tiles per instruction at 0.5 cyc/row -> 2x the fp16
    PE rate (~56us instead of ~112us of PE time).
  - fp8 noise gives logit errors ~0.25 abs, way over the rel-err budget for
    a sharply peaked softmax.  But softmax rows here concentrate on a few
    top entries (mass beyond top-32 < 2e-3), so we repair: per 64-wide
    s-window, DVE max8 selects the top-8 fp8 logits (128 candidates/row,
    a superset of everything that matters), match_replace knocks them down
    to -40 in the base row, dma_gather fetches those 128 enc columns in
    fp16, a small fp16 matmul recomputes their logits exactly, and the
    patched exp values are scattered into the output row over the base
    write (both on the in-order gpsimd dynamic DMA queue).  Host-simulated
    rel err of this exact scheme: 7.3e-4 (fp16 baseline: 6.8e-4).
  - Logit rows are produced replicated across partitions (the v-dot
    partition-sum matmul uses a [128,128] ones lhsT at the same cost as a
    [128,1] one), so the [1,1024] -> [128,64] window reshape for max8 is a
    single revisit-free SBUF DMA, and the dma_gather index tile comes out
    replicated across the 8 gpsimd cores for free.
  - energy tanh is fused on ScalarE: tanh(psum * 2^-16 + (Wh h + b)[o]) --
    the 2^-16 undoes the fp8 quantization scales (enc x16, W x4096).
  - softmax: exp(x - 40) with a constant shift (logits ~[-36, 37]); the
    match-replaced entries contribute exp(-80) ~= 0, repaired entries
    re-enter via the patch term, accum_out gives the denominator.
  - schedule: block (0,0) runs kp-outer so the PE consumes (wet8, enc8)
    DMA pairs as they land during the DMA-bound prefix; later blocks run
    mt-outer (1-2 PSUM banks live) with deferred work -- the previous
    chunk's ones-matmul, the previous row's selection + repair matmuls --
    injected between mt groups so the in-order PE queue never waits on
    DVE/DMA chains.
  - ~2us of junk matmuls pre-warm the PE HAM clock gate during the
    DMA prologue.
"""

from contextlib import ExitStack

import ml_dtypes
import numpy as np

import concourse.bacc as bacc
import concourse.mybir as mybir
import concourse.tile as tile
from concourse import bass_isa
from concourse.bass import IndirectOffsetOnAxis
from concourse.bass_utils import run_bass_kernel_spmd

H = 1024
B = 16
S = 1024
E = 2 * H
NCORES = 8
BL = B // NCORES        # 2 batch rows per core

PT = 128                # partition tile
NT = 512                # free-dim tile (one fp32 PSUM bank)
KP = E // (2 * PT)      # 8 DoubleRow K-pair tiles in the main matmul
KT = E // PT            # 16 fp16 K-tiles (repair matmul)
MT = H // PT            # 8 output-feature tiles
ST = S // NT            # 2 seq chunks
KT_H = H // PT          # 8 K-tiles for h_proj
W = 64                  # selection window width
NSEL = 128              # repaired columns per row (top-8 per window)

SC_E = 16.0             # fp8 quantization scales (center e4m3's range)
SC_W = 4096.0
UNSCALE = 1.0 / (SC_E * SC_W)

F32 = mybir.dt.float32
F32R = mybir.dt.float32r
F16 = mybir.dt.float16
F8 = mybir.dt.float8e4
U16 = mybir.dt.uint16
I16 = mybir.dt.int16
I32 = mybir.dt.int32
AF = mybir.ActivationFunctionType
DR = mybir.MatmulPerfMode.DoubleRow

# cf layout: [128, 16(bt) + 8(vt) + 1(nshift) + 16(ones) + 128(prep)]
CF_BT, CF_VT, CF_NS, CF_ONES = 0, KT_H * BL, KT_H * BL + MT, KT_H * BL + MT + 1
CF_PREP = CF_ONES + 16
CF_N = CF_PREP + 128

REPAIR = True


def build(repair=REPAIR, dbg=False):
    nc = bacc.Bacc("TRN2", target_bir_lowering=False, debug=False)

    enc8 = nc.dram_tensor("enc8", [BL, KP, PT, 2, S], F8, kind="ExternalInput").ap()
    wet8 = nc.dram_tensor("wet8", [KP, PT, 2, H], F8, kind="ExternalInput").ap()
    enc16 = nc.dram_tensor("enc16", [BL, S, E], F16, kind="ExternalInput").ap()
    wet16 = nc.dram_tensor("wet16", [KT, PT, H], F16, kind="ExternalInput").ap()
    wht = nc.dram_tensor("wht", [H, H], F16, kind="ExternalInput").ap()
    ht = nc.dram_tensor("ht", [PT, KT_H * BL], F16, kind="ExternalInput").ap()
    cf = nc.dram_tensor("cf", [PT, CF_N], F32, kind="ExternalInput").ap()
    vrep = nc.dram_tensor("vrep", [PT, MT * PT], F16, kind="ExternalInput").ap()
    pcol = nc.dram_tensor("pcol", [PT, 1], F32, kind="ExternalInput").ap()
    out = nc.dram_tensor("out", [1, BL * S], F32, kind="ExternalOutput").ap()
    hp_dram = nc.dram_tensor("hp_scratch", [BL, H], F32).ap()
    exl_dram = nc.dram_tensor("exl_scratch", [BL, NSEL], F16).ap()
    lg_dram = nc.dram_tensor("lg_scratch", [BL, S], F32).ap()
    if dbg:
        dbg_hpb = nc.dram_tensor("dbg_hpb", [PT, KT_H * BL], F32, kind="ExternalOutput").ap()
        dbg_lg = nc.dram_tensor("dbg_lg", [PT, S], F32, kind="ExternalOutput").ap()
        dbg_lgr = nc.dram_tensor("dbg_lgr", [PT, W], F32, kind="ExternalOutput").ap()
        dbg_mx = nc.dram_tensor("dbg_mx", [PT, 8], F32, kind="ExternalOutput").ap()
        dbg_gidxf = nc.dram_tensor("dbg_gidxf", [PT, 8], F32, kind="ExternalOutput").ap()
        dbg_exb = nc.dram_tensor("dbg_exb", [PT, W], F32, kind="ExternalOutput").ap()
        dbg_res = nc.dram_tensor("dbg_res", [16, W], F32, kind="ExternalOutput").ap()
        dbg_exl = nc.dram_tensor("dbg_exl", [16, NSEL], F16, kind="ExternalOutput").ap()
        dbg_gso = nc.dram_tensor("dbg_gso", [1, NSEL], F32, kind="ExternalOutput").ap()
        dbg_zz = nc.dram_tensor("dbg_zz", [1, 2], F32, kind="ExternalOutput").ap()
        dbg_G = nc.dram_tensor("dbg_G", [PT, KT * NSEL], F16, kind="ExternalOutput").ap()

    with tile.TileContext(nc) as tc, ExitStack() as ctx:
        constp = ctx.enter_context(tc.tile_pool(name="constp", bufs=1))
        wet8p = ctx.enter_context(tc.tile_pool(name="wet8p", bufs=KP))
        wet16p = ctx.enter_context(tc.tile_pool(name="wet16p", bufs=1))
        whtp = ctx.enter_context(tc.tile_pool(name="whtp", bufs=1))
        encp = ctx.enter_context(tc.tile_pool(name="encp", bufs=2 * KP))
        hpp = ctx.enter_context(tc.tile_pool(name="hpp", bufs=1))
        engp = ctx.enter_context(tc.tile_pool(name="engp", bufs=4))
        accp = ctx.enter_context(tc.tile_pool(name="accp", bufs=3))
        selp = ctx.enter_context(tc.tile_pool(name="selp", bufs=2))
        gp = ctx.enter_context(tc.tile_pool(name="gp", bufs=2))
        psp = ctx.enter_context(tc.tile_pool(name="psp", bufs=8, space="PSUM"))

        # ---- constants (ht first: the very first matmul needs it) -------
        ht_sb = constp.tile([PT, KT_H * BL], F16)
        nc.sync.dma_start(ht_sb[:], ht[:])
        cf_sb = constp.tile([PT, CF_N], F32)
        nc.sync.dma_start(cf_sb[:], cf[:])
        vrep_sb = constp.tile([PT, MT * PT], F16)
        nc.sync.dma_start(vrep_sb[:], vrep[:])
        pcol_sb = constp.tile([PT, 1], F32)
        nc.sync.dma_start(pcol_sb[:], pcol[:])


        bt_sb = cf_sb[:, CF_BT:CF_VT]
        vt_sb = cf_sb[:, CF_VT:CF_NS]
        nshift = cf_sb[:, CF_NS : CF_NS + 1]
        onesf = cf_sb[:, CF_ONES : CF_ONES + 16]
        prep = cf_sb[0:16, CF_PREP : CF_PREP + 128]

        # phase A weights stream on the gpsimd ring, off the main sync queue
        wht_sb = whtp.tile([PT, KT_H * H], F16, name="wht_sb")
        wht_v = wht_sb[:].rearrange("p (k o) -> p k o", k=KT_H)
        for kt in range(KT_H):
            nc.gpsimd.dma_start(wht_v[:, kt, :], wht[kt * PT : (kt + 1) * PT, :])
        # repair weights: big (4MB) but not needed until the first repair,
        # emitted on gpsimd after wht so the sync queue owns the prefix BW
        wet16_sb = wet16p.tile([PT, KT * H], F16, name="wet16_sb")
        wet16_v = wet16_sb[:].rearrange("p (k o) -> p k o", k=KT)
        for kt in range(KT):
            nc.gpsimd.dma_start(wet16_v[:, kt, :], wet16[kt])

        # HAM pre-warm: junk matmuls while the DMA prologue streams
        junk_ps = psp.tile([1, 2], F32, tag="ps", name="junk_ps2")
        for _ in range(60):
            nc.tensor.matmul(
                junk_ps[:], ht_sb[:, 0:1], ht_sb[:, 0:2],
                start=True, stop=True, skip_group_check=True,
            )

        # ---- phase A: hpb[o-tile][o, b] = (Wh @ h + attn_b) -------------
        php = [
            psp.tile([BL, NT], F32, tag="ps", name=f"php{oc}")
            for oc in range(H // NT)
        ]
        for kt in range(KT_H):
            for oc in range(H // NT):
                nc.tensor.matmul(
                    php[oc][:],
                    ht_sb[:, kt * BL : (kt + 1) * BL],
                    wht_v[:, kt, oc * NT : (oc + 1) * NT],
                    start=(kt == 0),
                    stop=(kt == KT_H - 1),
                )
        hp_sb = hpp.tile([BL, H], F32)
        for oc in range(H // NT):
            nc.scalar.copy(hp_sb[:, oc * NT : (oc + 1) * NT], php[oc][:])
        nc.gpsimd.dma_start(hp_dram[:], hp_sb[:])
        hpt_sb = hpp.tile([PT, KT_H * BL], F32, name="hpt_sb")
        for b in range(BL):
            nc.gpsimd.dma_start(
                hpt_sb[:].rearrange("p (m b) -> p m b", b=BL)[:, :, b],
                hp_dram[b].rearrange("(m p) -> p m", p=PT),
            )
        hpb_sb = hpp.tile([PT, KT_H * BL], F32, name="hpb_sb")
        nc.vector.tensor_add(hpb_sb[:], hpt_sb[:], bt_sb[:])
        if dbg:
            nc.gpsimd.dma_start(dbg_hpb[:], hpb_sb[:])

        # ---- main fp8 blocks -------------------------------------------
        wet8_tiles = [None] * KP

        def load_enc8(b, st):
            ts = []
            for kp in range(KP):
                t = encp.tile([PT, 2, NT], F8, name="enc_t")
                nc.sync.dma_start(
                    t[:], enc8[b, kp, :, :, st * NT : (st + 1) * NT]
                )
                ts.append(t)
            return ts

        def tanh_mt(pe_psum, b, mt):
            # fp16 tanh output feeds the PE v-dot (vrep lhsT) directly
            en = engp.tile([PT, NT], F16, name="en", tag="en")
            nc.scalar.activation(
                en[:], pe_psum[:], AF.Tanh,
                bias=hpb_sb[:, mt * BL + b : mt * BL + b + 1],
                scale=UNSCALE,
            )
            return en

        lgrow = {}
        for bb in range(BL):
            lgrow[bb] = selp.tile([1, S], F32, name=f"lgrow{bb}", tag=f"lgw{bb}")

        def dump_lg(pa, b, st):
            nc.scalar.copy(lgrow[b][:, st * NT : (st + 1) * NT], pa[0:1, :])

        def vd(pa, en, mt):
            nc.tensor.matmul(
                pa[:], vrep_sb[:, mt * PT : (mt + 1) * PT], en[:],
                start=(mt == 0), stop=(mt == MT - 1),
            )

        def block_kpouter(b, st, with_wet=False):
            pes = [
                psp.tile([PT, NT], F32, tag="ps", name=f"pes_{b}{st}_{mt}")
                for mt in range(MT)
            ]
            pa = psp.tile([PT, NT], F32, tag="ps", name=f"pa{b}{st}")
            ens = [None] * MT
            for kp in range(KP):
                if with_wet:
                    wt = wet8p.tile([PT, 2, H], F8, name="wet8_t")
                    nc.sync.dma_start(wt[:], wet8[kp])
                    wet8_tiles[kp] = wt
                t = encp.tile([PT, 2, NT], F8, name="enc_t")
                nc.sync.dma_start(t[:], enc8[b, kp, :, :, st * NT : (st + 1) * NT])
                for mt in range(MT):
                    nc.tensor.matmul(
                        pes[mt][:],
                        wet8_tiles[kp][:, :, mt * PT : (mt + 1) * PT],
                        t[:],
                        start=(kp == 0),
                        stop=(kp == KP - 1),
                        perf_mode=DR,
                    )
                    if kp == KP - 1:
                        # tanh chases the last kp group mt by mt
                        ens[mt] = tanh_mt(pes[mt], b, mt)
                        if mt >= 1:
                            vd(pa, ens[mt - 1], mt - 1)
            vd(pa, ens[MT - 1], MT - 1)
            dump_lg(pa, b, st)

        def block_mtouter(b, st, etiles, hooks=None):
            pa = psp.tile([PT, NT], F32, tag="ps", name=f"pa{b}{st}")
            prev_en = None
            for mt in range(MT):
                if hooks and mt in hooks:
                    for fn in hooks[mt]:
                        fn()
                pe = psp.tile([PT, NT], F32, tag="ps", name="pe")
                for kp in range(KP):
                    nc.tensor.matmul(
                        pe[:],
                        wet8_tiles[kp][:, :, mt * PT : (mt + 1) * PT],
                        etiles[kp][:],
                        start=(kp == 0),
                        stop=(kp == KP - 1),
                        perf_mode=DR,
                    )
                en = tanh_mt(pe, b, mt)
                if prev_en is not None:
                    vd(pa, prev_en, mt - 1)
                prev_en = en
            vd(pa, prev_en, MT - 1)
            dump_lg(pa, b, st)

        # ---- per-row selection + repair --------------------------------
        row_state = {}

        def select_pre(b):
            # [1,1024] -> [16,64] windows, one SBUF->SBUF DMA; all selection
            # DVE work on 16 partitions
            q = nc.sync if b == 1 else nc.gpsimd
            lgr = selp.tile([16, W], F32, name="lgr", tag=f"lgr{b}")
            q.dma_start(
                lgr[:], lgrow[b][:].rearrange("o (q f) -> o q f", q=16)
            )
            mx = selp.tile([16, 8], F32, name="mx", tag=f"mx{b}")
            nc.vector.max(mx[:], lgr[:])
            mi = selp.tile([16, 8], U16, name="mi", tag=f"mi{b}")
            nc.vector.max_index(mi[:], mx[:], lgr[:])
            rmax = selp.tile([16, 1], F32, name="rmax", tag=f"rmax{b}")
            nc.gpsimd.partition_all_reduce(
                rmax[:], mx[:, 0:1], 16, bass_isa.ReduceOp.max
            )
            nsh = selp.tile([16, 1], F32, name="nsh", tag=f"nsh{b}")
            nc.vector.tensor_scalar_mul(nsh[:], rmax[:], -1.0)
            gidxf = selp.tile([16, 8], F32, name="gidxf", tag=f"gidxf{b}")
            nc.vector.tensor_scalar_add(gidxf[:], mi[:], pcol_sb[0:16, 0:1])
            if repair:
                lgrep = selp.tile([16, W], F32, name="lgrep", tag=f"lgrep{b}")
                nc.vector.match_replace(lgrep[:], mx[:], lgr[:], -100.0)
            else:
                lgrep = lgr
            exb = selp.tile([16, W], F32, name="exb", tag=f"exb{b}")
            zb = selp.tile([16, 1], F32, name="zb", tag=f"zb{b}")
            nc.scalar.activation(
                exb[:], lgrep[:], AF.Exp, bias=nsh[:, 0:1], accum_out=zb[:]
            )
            row_state[b] = dict(exb=exb, zb=zb, mi=mi, nsh=nsh, gidxf=gidxf)

        def select_fin(b):
            # replicate the gather indices to 128 partitions on the PE, then
            # fetch the selected enc columns in fp16
            st_ = row_state[b]
            gps = psp.tile([PT, 8], F32, tag="ps", name="gps")
            nc.tensor.matmul(gps[:], prep, st_["gidxf"][:], start=True, stop=True)
            gidx = selp.tile([PT, 8], I16, name="gidx", tag=f"gidx{b}")
            nc.vector.tensor_copy(gidx[:], gps[:])
            G = gp.tile([PT, KT * NSEL], F16, name="G", tag=f"G{b}")
            if repair:
                nc.gpsimd.dma_gather(
                    G[:].rearrange("p (k c) -> p k c", k=KT),
                    enc16[b],
                    gidx[:],
                    NSEL,
                    NSEL,
                    E,
                    transpose=True,
                )
            st_["G"] = G

        def repair_matmuls(b):
            st_ = row_state[b]
            Gv = st_["G"][:].rearrange("p (k c) -> p k c", k=KT)
            prs = []
            for mt in range(MT):
                pr = psp.tile([PT, NSEL], F32, tag="ps", name=f"pr{b}{mt}")
                for kt in range(KT):
                    nc.tensor.matmul(
                        pr[:],
                        wet16_v[:, kt, mt * PT : (mt + 1) * PT],
                        Gv[:, kt, :],
                        start=(kt == 0),
                        stop=(kt == KT - 1),
                    )
                prs.append(pr)
            st_["prs"] = prs

        def repair_rest(b):
            st_ = row_state[b]
            exl = None
            if repair:
                accr = accp.tile([PT, NSEL], F32, name="accr", tag="accr")
                for mt in range(MT):
                    enr = engp.tile([PT, NSEL], F32, name="enr", tag="enr")
                    nc.scalar.activation(
                        enr[:], st_["prs"][mt][:], AF.Tanh,
                        bias=hpb_sb[:, mt * BL + b : mt * BL + b + 1],
                    )
                    if mt == 0:
                        nc.vector.tensor_scalar_mul(accr[:], enr[:], vt_sb[:, 0:1])
                    else:
                        nc.vector.scalar_tensor_tensor(
                            accr[:], enr[:], vt_sb[:, mt : mt + 1], accr[:],
                            mybir.AluOpType.mult, mybir.AluOpType.add,
                        )
                # exact logits, replicated over 16 partitions
                lex = psp.tile([16, NSEL], F32, tag="ps", name="lex")
                nc.tensor.matmul(
                    lex[:], onesf[:, 0:16], accr[:], start=True, stop=True)
                exl = selp.tile([16, NSEL], F16, name="exl", tag=f"exl{b}")
                zr = selp.tile([16, 1], F32, name="zr", tag=f"zr{b}")
                nc.scalar.activation(
                    exl[:], lex[:], AF.Exp, bias=st_["nsh"][:, 0:1],
                    accum_out=zr[:],
                )
            # denominator: window partials + repaired sum, one PSUM group
            zps = psp.tile([1, 1], F32, tag="ps", name="zps")
            nc.tensor.matmul(
                zps[:], onesf[0:16, 0:1], st_["zb"][:],
                start=True, stop=not repair,
            )
            if repair:
                nc.tensor.matmul(
                    zps[:], onesf[0:1, 0:1], zr[0:1, 0:1], start=False, stop=True
                )
            rs = selp.tile([1, 1], F32, name="rs", tag=f"rs{b}")
            nc.vector.reciprocal(rs[:], zps[:])
            rzp = psp.tile([16, 1], F32, tag="ps", name="rzp")
            nc.tensor.matmul(rzp[:], onesf[0:1, 0:16], rs[:], start=True, stop=True)
            rz16 = selp.tile([16, 1], F32, name="rz16", tag=f"rz16{b}")
            nc.scalar.copy(rz16[:], rzp[:])
            # normalized base row + patch values
            exbp = st_["exb"][:]
            if repair:
                # exl row (slot order c = j*16+p) -> [16,8] via DRAM reorder,
                # then a per-partition SBUF scatter into the 64-wide windows
                nc.gpsimd.dma_start(exl_dram[b : b + 1, :], exl[0:1, :])
                exv16 = selp.tile([16, 8], F16, name="exv16", tag=f"exv16{b}")
                nc.gpsimd.dma_start(
                    exv16[:], exl_dram[b].rearrange("(j p) -> p j", p=16)
                )
                pg = selp.tile([16, W], F16, name="pg", tag=f"pg{b}")
                nc.gpsimd.local_scatter(
                    pg[:], exv16[:], st_["mi"][:].bitcast(I16), 16, W, 8
                )
                exbsum = selp.tile([16, W], F32, name="exbsum", tag=f"exbs{b}")
                nc.vector.tensor_add(exbsum[:], exbp, pg[:])
                exbp = exbsum[:]
            res = selp.tile([16, W], F32, name="res", tag=f"res{b}")
            nc.vector.tensor_scalar_mul(res[:], exbp, rz16[:, 0:1])
            if dbg and b == 0:
                nc.gpsimd.dma_start(dbg_res[:], res[:])
                if repair:
                    nc.gpsimd.dma_start(dbg_exl[:], exl[:])
                    nc.gpsimd.dma_start(dbg_G[:], st_["G"][:])
                zz = selp.tile([1, 2], F32, name="zz", tag="zz")
                nc.vector.tensor_copy(zz[:, 0:1], zps[:])
                nc.vector.tensor_copy(zz[:, 1:2], rs[:])
                nc.gpsimd.dma_start(dbg_zz[:], zz[:])
            nc.sync.dma_start(
                out[0, b * S : (b + 1) * S].rearrange("(q f) -> q f", q=16),
                res[:],
            )

        # ---- schedule ---------------------------------------------------
        block_kpouter(0, 0, with_wet=True)
        et01 = load_enc8(0, 1)
        block_mtouter(0, 1, et01)
        et10 = load_enc8(1, 0)
        block_mtouter(
            1, 0, et10,
            hooks={0: [lambda: select_pre(0)], 4: [lambda: select_fin(0)]},
        )
        et11 = load_enc8(1, 1)
        hooks11 = {}
        if repair:
            hooks11[1] = [lambda: repair_matmuls(0)]
            hooks11[5] = [lambda: repair_rest(0)]
        else:
            hooks11[1] = [lambda: repair_rest(0)]
        block_mtouter(1, 1, et11, hooks=hooks11)
        select_pre(1)
        select_fin(1)
        if repair:
            repair_matmuls(1)
        repair_rest(1)

    nc.compile()
    return nc


_NC_CACHE = {}


def _get_nc(repair=REPAIR, dbg=False):
    key = (repair, dbg)
    if key not in _NC_CACHE:
        _NC_CACHE[key] = build(repair, dbg)
    return _NC_CACHE[key]


def make_in_maps(hidden_state, encoder_outputs, attn_w, attn_b, v):
    hidden_state = np.asarray(hidden_state, dtype=np.float32)
    encoder_outputs = np.asarray(encoder_outputs, dtype=np.float32)
    attn_w = np.asarray(attn_w, dtype=np.float32)
    attn_b = np.asarray(attn_b, dtype=np.float32)
    v = np.asarray(v, dtype=np.float32)

    w_e_t = np.ascontiguousarray(attn_w[:, H:].T)          # [E, H]
    wht_t = np.ascontiguousarray(attn_w[:, :H].T).astype(np.float16)
    encb = encoder_outputs.transpose(1, 0, 2)              # [B, S, E]

    # fp8 operands (scaled into e4m3's sweet spot)
    wet8_t = np.asarray(w_e_t * SC_W, dtype=ml_dtypes.float8_e4m3)
    wet8_t = np.ascontiguousarray(
        wet8_t.reshape(KP, 2, PT, H).transpose(0, 2, 1, 3))   # [KP,128,2,H]
    wet16_t = np.ascontiguousarray(w_e_t.reshape(KT, PT, H)).astype(np.float16)

    bt_t = np.repeat(
        attn_b.reshape(MT, PT).T[:, :, None], BL, axis=2
    ).reshape(PT, MT * BL)
    vt_t = np.ascontiguousarray(v.reshape(MT, PT).T)
    prep_t = (np.arange(128)[None, :] % 16 ==
              np.arange(PT)[:, None]).astype(np.float32)
    cf_t = np.concatenate(
        [bt_t, vt_t, np.full((PT, 1), -40.0, np.float32),
         np.ones((PT, 16), np.float32), prep_t], axis=1,
    ).astype(np.float32)
    pcol_t = ((np.arange(PT) % 16) * W).astype(np.float32).reshape(PT, 1)
    vrep_t = np.ascontiguousarray(
        np.repeat(v.reshape(MT, PT, 1), PT, axis=2).transpose(1, 0, 2)
        .reshape(PT, MT * PT)).astype(np.float16)

    in_maps = []
    for i in range(NCORES):
        rows = slice(i * BL, (i + 1) * BL)
        enc_c = encb[rows]                                 # [BL, S, E]
        encT = enc_c.transpose(0, 2, 1)                    # [BL, E, S]
        enc8_t = np.asarray(encT * SC_E, dtype=ml_dtypes.float8_e4m3)
        enc8_t = np.ascontiguousarray(
            enc8_t.reshape(BL, KP, 2, PT, S).transpose(0, 1, 3, 2, 4))
        in_maps.append(
            {
                "enc8": enc8_t,
                "wet8": wet8_t,
                "enc16": np.ascontiguousarray(enc_c).astype(np.float16),
                "wet16": wet16_t,
                "wht": wht_t,
                "ht": np.ascontiguousarray(
                    hidden_state[rows].T.reshape(KT_H, PT, BL)
                    .transpose(1, 0, 2).reshape(PT, KT_H * BL)
                ).astype(np.float16),
                "cf": cf_t,
                "vrep": vrep_t,
                "pcol": pcol_t,
            }
        )
    return in_maps


def run(inputs, trace=False, compute_dtype=None, dbg=False, **spmd_kwargs):
    nc = _get_nc(dbg=dbg)
    in_maps = make_in_maps(**inputs)
    res = run_bass_kernel_spmd(
        nc, in_maps, core_ids=list(range(NCORES)), trace=trace, **spmd_kwargs
    )
    out = np.concatenate(
        [res.results[i]["out"].reshape(BL, S) for i in range(NCORES)], axis=0
    )
    return out.astype(np.float32), res


def kernel(**inputs):
    out, _ = run(inputs, trace=False)
    return out


# revision 34
# speedup vs baseline: 1.2345x; 1.1096x over previous
"""Trainium2 Bass kernel for nn_Attention (additive/Bahdanau-style attention).

Math (reference):
    enc [S,B,2H] -> [B,S,2H]
    energy  = tanh(h @ Wh^T + enc @ We^T + b)    # [B,S,H]
    logits  = energy . v                         # [B,S]
    out     = softmax(logits, axis=S)            # [B,S]

Sharding: data-parallel over batch. B=16 rows over 8 NeuronCores -> 2 rows
per core; attn weights replicated. No collectives.

fp8 + exact-repair design (per core):
  - The big matmul e_projT = We^T.T @ encT (K=2048) runs in fp8(e4m3)
    DoubleRow mode: 2 K-tiles per instruction at 0.5 cyc/row -> 2x the fp16
    PE rate (~56us instead of ~112us of PE time).
  - fp8 noise gives logit errors ~0.25 abs, way over the rel-err budget for
    a sharply peaked softmax.  But softmax rows here concentrate on a few
    top entries (mass beyond top-32 < 2e-3), so we repair: per 64-wide
    s-window, DVE max8 selects the top-8 fp8 logits (128 candidates/row,
    a superset of everything that matters), match_replace knocks them down
    to -40 in the base row, dma_gather fetches those 128 enc columns in
    fp16, a small fp16 matmul recomputes their logits exactly, and the
    patched exp values are scattered into the output row over the base
    write (both on the in-order gpsimd dynamic DMA queue).  Host-simulated
    rel err of this exact scheme: 7.3e-4 (fp16 baseline: 6.8e-4).
  - Logit rows are produced replicated across partitions (the v-dot
    partition-sum matmul uses a [128,128] ones lhsT at the same cost as a
    [128,1] one), so the [1,1024] -> [128,64] window reshape for max8 is a
    single revisit-free SBUF DMA, and the dma_gather index tile comes out
    replicated across the 8 gpsimd cores for free.
  - energy tanh is fused on ScalarE: tanh(psum * 2^-16 + (Wh h + b)[o]) --
    the 2^-16 undoes the fp8 quantization scales (enc x16, W x4096).
  - softmax: exp(x - 40) with a constant shift (logits ~[-36, 37]); the
    match-replaced entries contribute exp(-80) ~= 0, repaired entries
    re-enter via the patch term, accum_out gives the denominator.
  - schedule: block (0,0) runs kp-outer so the PE consumes (wet8, enc8)
    DMA pairs as they land during the DMA-bound prefix; later blocks run
    mt-outer (1-2 PSUM banks live) with deferred work -- the previous
    chunk's ones-matmul, the previous row's selection + repair matmuls --
    injected between mt groups so the in-order PE queue never waits on
    DVE/DMA chains.
  - ~2us of junk matmuls pre-warm the PE HAM clock gate during the
    DMA prologue.
"""

from contextlib import ExitStack

import ml_dtypes
import numpy as np

import concourse.bacc as bacc
import concourse.mybir as mybir
import concourse.tile as tile
from concourse import bass_isa
from concourse.bass import IndirectOffsetOnAxis
from concourse.bass_utils import run_bass_kernel_spmd

H = 1024
B = 16
S = 1024
E = 2 * H
NCORES = 8
BL = B // NCORES        # 2 batch rows per core

PT = 128                # partition tile
NT = 512                # free-dim tile (one fp32 PSUM bank)
KP = E // (2 * PT)      # 8 DoubleRow K-pair tiles in the main matmul
KT = E // PT            # 16 fp16 K-tiles (repair matmul)
MT = H // PT            # 8 output-feature tiles
ST = S // NT            # 2 seq chunks
KT_H = H // PT          # 8 K-tiles for h_proj
W = 64                  # selection window width
NSEL = 128              # repaired columns per row (top-8 per window)

SC_E = 16.0             # fp8 quantization scales (center e4m3's range)
SC_W = 4096.0
UNSCALE = 1.0 / (SC_E * SC_W)

F32 = mybir.dt.float32
F32R = mybir.dt.float32r
F16 = mybir.dt.float16
F8 = mybir.dt.float8e4
U16 = mybir.dt.uint16
I16 = mybir.dt.int16
I32 = mybir.dt.int32
AF = mybir.ActivationFunctionType
DR = mybir.MatmulPerfMode.DoubleRow

# cf layout: [128, 16(bt) + 8(vt) + 1(nshift) + 16(ones) + 128(prep)]
CF_BT, CF_VT, CF_NS, CF_ONES = 0, KT_H * BL, KT_H * BL + MT, KT_H * BL + MT + 1
CF_PREP = CF_ONES + 16
CF_N = CF_PREP + 128

REPAIR = True


def build(repair=REPAIR, dbg=False):
    nc = bacc.Bacc("TRN2", target_bir_lowering=False, debug=False)

    enc8 = nc.dram_tensor("enc8", [BL, KP, PT, 2, S], F8, kind="ExternalInput").ap()
    wet8 = nc.dram_tensor("wet8", [KP, PT, 2, H], F8, kind="ExternalInput").ap()
    enc16 = nc.dram_tensor("enc16", [BL, S, E], F16, kind="ExternalInput").ap()
    wet16 = nc.dram_tensor("wet16", [KT, PT, H], F16, kind="ExternalInput").ap()
    wht = nc.dram_tensor("wht", [H, H], F16, kind="ExternalInput").ap()
    ht = nc.dram_tensor("ht", [PT, KT_H * BL], F16, kind="ExternalInput").ap()
    cf = nc.dram_tensor("cf", [PT, CF_N], F32, kind="ExternalInput").ap()
    vrep = nc.dram_tensor("vrep", [PT, MT * PT], F16, kind="ExternalInput").ap()
    pcol = nc.dram_tensor("pcol", [PT, 1], F32, kind="ExternalInput").ap()
    out = nc.dram_tensor("out", [1, BL * S], F32, kind="ExternalOutput").ap()
    hp_dram = nc.dram_tensor("hp_scratch", [BL, H], F32).ap()
    exl_dram = nc.dram_tensor("exl_scratch", [BL, NSEL], F16).ap()
    lg_dram = nc.dram_tensor("lg_scratch", [BL, S], F32).ap()
    if dbg:
        dbg_hpb = nc.dram_tensor("dbg_hpb", [PT, KT_H * BL], F32, kind="ExternalOutput").ap()
        dbg_lg = nc.dram_tensor("dbg_lg", [PT, S], F32, kind="ExternalOutput").ap()
        dbg_lgr = nc.dram_tensor("dbg_lgr", [PT, W], F32, kind="ExternalOutput").ap()
        dbg_mx = nc.dram_tensor("dbg_mx", [PT, 8], F32, kind="ExternalOutput").ap()
        dbg_gidxf = nc.dram_tensor("dbg_gidxf", [PT, 8], F32, kind="ExternalOutput").ap()
        dbg_exb = nc.dram_tensor("dbg_exb", [PT, W], F32, kind="ExternalOutput").ap()
        dbg_res = nc.dram_tensor("dbg_res", [16, W], F32, kind="ExternalOutput").ap()
        dbg_exl = nc.dram_tensor("dbg_exl", [16, NSEL], F16, kind="ExternalOutput").ap()
        dbg_gso = nc.dram_tensor("dbg_gso", [1, NSEL], F32, kind="ExternalOutput").ap()
        dbg_zz = nc.dram_tensor("dbg_zz", [1, 2], F32, kind="ExternalOutput").ap()
        dbg_G = nc.dram_tensor("dbg_G", [PT, KT * NSEL], F16, kind="ExternalOutput").ap()

    with tile.TileContext(nc) as tc, ExitStack() as ctx:
        constp = ctx.enter_context(tc.tile_pool(name="constp", bufs=1))
        wet8p = ctx.enter_context(tc.tile_pool(name="wet8p", bufs=KP))
        wet16p = ctx.enter_context(tc.tile_pool(name="wet16p", bufs=1))
        whtp = ctx.enter_context(tc.tile_pool(name="whtp", bufs=1))
        encp = ctx.enter_context(tc.tile_pool(name="encp", bufs=2 * KP))
        hpp = ctx.enter_context(tc.tile_pool(name="hpp", bufs=1))
        engp = ctx.enter_context(tc.tile_pool(name="engp", bufs=10))
        accp = ctx.enter_context(tc.tile_pool(name="accp", bufs=3))
        selp = ctx.enter_context(tc.tile_pool(name="selp", bufs=2))
        gp = ctx.enter_context(tc.tile_pool(name="gp", bufs=2))
        psp = ctx.enter_context(tc.tile_pool(name="psp", bufs=8, space="PSUM"))

        # ---- constants (ht first: the very first matmul needs it) -------
        ht_sb = constp.tile([PT, KT_H * BL], F16)
        nc.sync.dma_start(ht_sb[:], ht[:])
        cf_sb = constp.tile([PT, CF_N], F32)
        nc.sync.dma_start(cf_sb[:], cf[:])
        vrep_sb = constp.tile([PT, MT * PT], F16)
        nc.sync.dma_start(vrep_sb[:], vrep[:])
        pcol_sb = constp.tile([PT, 1], F32)
        nc.sync.dma_start(pcol_sb[:], pcol[:])


        bt_sb = cf_sb[:, CF_BT:CF_VT]
        vt_sb = cf_sb[:, CF_VT:CF_NS]
        nshift = cf_sb[:, CF_NS : CF_NS + 1]
        onesf = cf_sb[:, CF_ONES : CF_ONES + 16]
        prep = cf_sb[0:16, CF_PREP : CF_PREP + 128]

        # phase A weights stream on the gpsimd ring, off the main sync queue
        wht_sb = whtp.tile([PT, KT_H * H], F16, name="wht_sb")
        wht_v = wht_sb[:].rearrange("p (k o) -> p k o", k=KT_H)
        for kt in range(KT_H):
            nc.gpsimd.dma_start(wht_v[:, kt, :], wht[kt * PT : (kt + 1) * PT, :])
        # repair weights: big (4MB) but not needed until the first repair,
        # emitted on gpsimd after wht so the sync queue owns the prefix BW
        wet16_sb = wet16p.tile([PT, KT * H], F16, name="wet16_sb")
        wet16_v = wet16_sb[:].rearrange("p (k o) -> p k o", k=KT)
        for kt in range(KT):
            nc.gpsimd.dma_start(wet16_v[:, kt, :], wet16[kt])

        # HAM pre-warm: junk matmuls while the DMA prologue streams
        junk_ps = psp.tile([1, 2], F32, tag="ps", name="junk_ps2")
        for _ in range(60):
            nc.tensor.matmul(
                junk_ps[:], ht_sb[:, 0:1], ht_sb[:, 0:2],
                start=True, stop=True, skip_group_check=True,
            )

        # ---- phase A: hpb[o-tile][o, b] = (Wh @ h + attn_b) -------------
        php = [
            psp.tile([BL, NT], F32, tag="ps", name=f"php{oc}")
            for oc in range(H // NT)
        ]
        for kt in range(KT_H):
            for oc in range(H // NT):
                nc.tensor.matmul(
                    php[oc][:],
                    ht_sb[:, kt * BL : (kt + 1) * BL],
                    wht_v[:, kt, oc * NT : (oc + 1) * NT],
                    start=(kt == 0),
                    stop=(kt == KT_H - 1),
                )
        hp_sb = hpp.tile([BL, H], F32)
        for oc in range(H // NT):
            nc.scalar.copy(hp_sb[:, oc * NT : (oc + 1) * NT], php[oc][:])
        nc.gpsimd.dma_start(hp_dram[:], hp_sb[:])
        hpt_sb = hpp.tile([PT, KT_H * BL], F32, name="hpt_sb")
        for b in range(BL):
            nc.gpsimd.dma_start(
                hpt_sb[:].rearrange("p (m b) -> p m b", b=BL)[:, :, b],
                hp_dram[b].rearrange("(m p) -> p m", p=PT),
            )
        hpb_sb = hpp.tile([PT, KT_H * BL], F32, name="hpb_sb")
        nc.vector.tensor_add(hpb_sb[:], hpt_sb[:], bt_sb[:])
        if dbg:
            nc.gpsimd.dma_start(dbg_hpb[:], hpb_sb[:])

        # ---- main fp8 blocks -------------------------------------------
        wet8_tiles = [None] * KP

        def load_enc8(b, st):
            ts = []
            for kp in range(KP):
                t = encp.tile([PT, 2, NT], F8, name="enc_t")
                nc.sync.dma_start(
                    t[:], enc8[b, kp, :, :, st * NT : (st + 1) * NT]
                )
                ts.append(t)
            return ts

        def tanh_mt(pe_psum, b, mt):
            # fp16 tanh output feeds the PE v-dot (vrep lhsT) directly
            en = engp.tile([PT, NT], F16, name="en", tag="en")
            nc.scalar.activation(
                en[:], pe_psum[:], AF.Tanh,
                bias=hpb_sb[:, mt * BL + b : mt * BL + b + 1],
                scale=UNSCALE,
            )
            return en

        lgrow = {}
        for bb in range(BL):
            lgrow[bb] = selp.tile([1, S], F32, name=f"lgrow{bb}", tag=f"lgw{bb}")

        def dump_lg(pa, b, st):
            nc.scalar.copy(lgrow[b][:, st * NT : (st + 1) * NT], pa[0:1, :])

        def vd(pa, en, mt):
            nc.tensor.matmul(
                pa[:], vrep_sb[:, mt * PT : (mt + 1) * PT], en[:],
                start=(mt == 0), stop=(mt == MT - 1),
            )

        def block_kpouter(b, st, with_wet=False):
            pes = [
                psp.tile([PT, NT], F32, tag="ps", name=f"pes_{b}{st}_{mt}")
                for mt in range(MT)
            ]
            pa = psp.tile([PT, NT], F32, tag="ps", name=f"pa{b}{st}")
            ens = [None] * MT
            for kp in range(KP):
                if with_wet:
                    wt = wet8p.tile([PT, 2, H], F8, name="wet8_t")
                    nc.sync.dma_start(wt[:], wet8[kp])
                    wet8_tiles[kp] = wt
                t = encp.tile([PT, 2, NT], F8, name="enc_t")
                nc.sync.dma_start(t[:], enc8[b, kp, :, :, st * NT : (st + 1) * NT])
                for mt in range(MT):
                    nc.tensor.matmul(
                        pes[mt][:],
                        wet8_tiles[kp][:, :, mt * PT : (mt + 1) * PT],
                        t[:],
                        start=(kp == 0),
                        stop=(kp == KP - 1),
                        perf_mode=DR,
                    )
                    if kp == KP - 1:
                        ens[mt] = tanh_mt(pes[mt], b, mt)
            for mt in range(MT):
                vd(pa, ens[mt], mt)
            dump_lg(pa, b, st)

        def block_mtouter(b, st, etiles, hooks=None):
            pa = psp.tile([PT, NT], F32, tag="ps", name=f"pa{b}{st}")
            ens = [None] * MT
            for mt in range(MT):
                if hooks and mt in hooks:
                    for fn in hooks[mt]:
                        fn()
                pe = psp.tile([PT, NT], F32, tag="ps", name="pe")
                for kp in range(KP):
                    nc.tensor.matmul(
                        pe[:],
                        wet8_tiles[kp][:, :, mt * PT : (mt + 1) * PT],
                        etiles[kp][:],
                        start=(kp == 0),
                        stop=(kp == KP - 1),
                        perf_mode=DR,
                    )
                ens[mt] = tanh_mt(pe, b, mt)
            # batched v-dot keeps the DR weight-load pipeline undisturbed
            for mt in range(MT):
                vd(pa, ens[mt], mt)
            dump_lg(pa, b, st)

        # ---- per-row selection + repair --------------------------------
        row_state = {}

        def select_pre(b):
            # [1,1024] -> [16,64] windows, one SBUF->SBUF DMA; all selection
            # DVE work on 16 partitions
            q = nc.sync if b == 1 else nc.gpsimd
            lgr = selp.tile([16, W], F32, name="lgr", tag=f"lgr{b}")
            q.dma_start(
                lgr[:], lgrow[b][:].rearrange("o (q f) -> o q f", q=16)
            )
            mx = selp.tile([16, 8], F32, name="mx", tag=f"mx{b}")
            nc.vector.max(mx[:], lgr[:])
            mi = selp.tile([16, 8], U16, name="mi", tag=f"mi{b}")
            nc.vector.max_index(mi[:], mx[:], lgr[:])
            rmax = selp.tile([16, 1], F32, name="rmax", tag=f"rmax{b}")
            nc.gpsimd.partition_all_reduce(
                rmax[:], mx[:, 0:1], 16, bass_isa.ReduceOp.max
            )
            nsh = selp.tile([16, 1], F32, name="nsh", tag=f"nsh{b}")
            nc.vector.tensor_scalar_mul(nsh[:], rmax[:], -1.0)
            gidxf = selp.tile([16, 8], F32, name="gidxf", tag=f"gidxf{b}")
            nc.vector.tensor_scalar_add(gidxf[:], mi[:], pcol_sb[0:16, 0:1])
            if repair:
                lgrep = selp.tile([16, W], F32, name="lgrep", tag=f"lgrep{b}")
                nc.vector.match_replace(lgrep[:], mx[:], lgr[:], -100.0)
            else:
                lgrep = lgr
            exb = selp.tile([16, W], F32, name="exb", tag=f"exb{b}")
            zb = selp.tile([16, 1], F32, name="zb", tag=f"zb{b}")
            nc.scalar.activation(
                exb[:], lgrep[:], AF.Exp, bias=nsh[:, 0:1], accum_out=zb[:]
            )
            row_state[b] = dict(exb=exb, zb=zb, mi=mi, nsh=nsh, gidxf=gidxf)

        def select_fin(b):
            # replicate the gather indices to 128 partitions on the PE, then
            # fetch the selected enc columns in fp16
            st_ = row_state[b]
            gps = psp.tile([PT, 8], F32, tag="ps", name="gps")
            nc.tensor.matmul(gps[:], prep, st_["gidxf"][:], start=True, stop=True)
            gidx = selp.tile([PT, 8], I16, name="gidx", tag=f"gidx{b}")
            nc.vector.tensor_copy(gidx[:], gps[:])
            G = gp.tile([PT, KT * NSEL], F16, name="G", tag=f"G{b}")
            if repair:
                nc.gpsimd.dma_gather(
                    G[:].rearrange("p (k c) -> p k c", k=KT),
                    enc16[b],
                    gidx[:],
                    NSEL,
                    NSEL,
                    E,
                    transpose=True,
                )
            st_["G"] = G

        def repair_matmuls(b):
            st_ = row_state[b]
            Gv = st_["G"][:].rearrange("p (k c) -> p k c", k=KT)
            prs = []
            for mt in range(MT):
                pr = psp.tile([PT, NSEL], F32, tag="ps", name=f"pr{b}{mt}")
                for kt in range(KT):
                    nc.tensor.matmul(
                        pr[:],
                        wet16_v[:, kt, mt * PT : (mt + 1) * PT],
                        Gv[:, kt, :],
                        start=(kt == 0),
                        stop=(kt == KT - 1),
                    )
                prs.append(pr)
            st_["prs"] = prs

        def repair_rest(b):
            st_ = row_state[b]
            exl = None
            if repair:
                accr = accp.tile([PT, NSEL], F32, name="accr", tag="accr")
                for mt in range(MT):
                    enr = engp.tile([PT, NSEL], F32, name="enr", tag="enr")
                    nc.scalar.activation(
                        enr[:], st_["prs"][mt][:], AF.Tanh,
                        bias=hpb_sb[:, mt * BL + b : mt * BL + b + 1],
                    )
                    if mt == 0:
                        nc.vector.tensor_scalar_mul(accr[:], enr[:], vt_sb[:, 0:1])
                    else:
                        nc.vector.scalar_tensor_tensor(
                            accr[:], enr[:], vt_sb[:, mt : mt + 1], accr[:],
                            mybir.AluOpType.mult, mybir.AluOpType.add,
                        )
                # exact logits, replicated over 16 partitions
                lex = psp.tile([16, NSEL], F32, tag="ps", name="lex")
                nc.tensor.matmul(
                    lex[:], onesf[:, 0:16], accr[:], start=True, stop=True)
                exl = selp.tile([16, NSEL], F16, name="exl", tag=f"exl{b}")
                zr = selp.tile([16, 1], F32, name="zr", tag=f"zr{b}")
                nc.scalar.activation(
                    exl[:], lex[:], AF.Exp, bias=st_["nsh"][:, 0:1],
                    accum_out=zr[:],
                )
            # denominator: window partials + repaired sum, one PSUM group
            zps = psp.tile([1, 1], F32, tag="ps", name="zps")
            nc.tensor.matmul(
                zps[:], onesf[0:16, 0:1], st_["zb"][:],
                start=True, stop=not repair,
            )
            if repair:
                nc.tensor.matmul(
                    zps[:], onesf[0:1, 0:1], zr[0:1, 0:1], start=False, stop=True
                )
            rs = selp.tile([1, 1], F32, name="rs", tag=f"rs{b}")
            nc.vector.reciprocal(rs[:], zps[:])
            rzp = psp.tile([16, 1], F32, tag="ps", name="rzp")
            nc.tensor.matmul(rzp[:], onesf[0:1, 0:16], rs[:], start=True, stop=True)
            rz16 = selp.tile([16, 1], F32, name="rz16", tag=f"rz16{b}")
            nc.scalar.copy(rz16[:], rzp[:])
            # normalized base row + patch values
            exbp = st_["exb"][:]
            if repair:
                # exl row (slot order c = j*16+p) -> [16,8] via DRAM reorder,
                # then a per-partition SBUF scatter into the 64-wide windows
                nc.gpsimd.dma_start(exl_dram[b : b + 1, :], exl[0:1, :])
                exv16 = selp.tile([16, 8], F16, name="exv16", tag=f"exv16{b}")
                nc.gpsimd.dma_start(
                    exv16[:], exl_dram[b].rearrange("(j p) -> p j", p=16)
                )
                pg = selp.tile([16, W], F16, name="pg", tag=f"pg{b}")
                nc.gpsimd.local_scatter(
                    pg[:], exv16[:], st_["mi"][:].bitcast(I16), 16, W, 8
                )
                exbsum = selp.tile([16, W], F32, name="exbsum", tag=f"exbs{b}")
                nc.vector.tensor_add(exbsum[:], exbp, pg[:])
                exbp = exbsum[:]
            res = selp.tile([16, W], F32, name="res", tag=f"res{b}")
            nc.vector.tensor_scalar_mul(res[:], exbp, rz16[:, 0:1])
            if dbg and b == 0:
                nc.gpsimd.dma_start(dbg_res[:], res[:])
                if repair:
                    nc.gpsimd.dma_start(dbg_exl[:], exl[:])
                    nc.gpsimd.dma_start(dbg_G[:], st_["G"][:])
                zz = selp.tile([1, 2], F32, name="zz", tag="zz")
                nc.vector.tensor_copy(zz[:, 0:1], zps[:])
                nc.vector.tensor_copy(zz[:, 1:2], rs[:])
                nc.gpsimd.dma_start(dbg_zz[:], zz[:])
            nc.sync.dma_start(
                out[0, b * S : (b + 1) * S].rearrange("(q f) -> q f", q=16),
                res[:],
            )

        # ---- schedule ---------------------------------------------------
        block_kpouter(0, 0, with_wet=True)
        et01 = load_enc8(0, 1)
        block_mtouter(0, 1, et01)
        et10 = load_enc8(1, 0)
        block_mtouter(
            1, 0, et10,
            hooks={0: [lambda: select_pre(0)], 4: [lambda: select_fin(0)]},
        )
        et11 = load_enc8(1, 1)
        hooks11 = {}
        if repair:
            hooks11[1] = [lambda: repair_matmuls(0)]
            hooks11[5] = [lambda: repair_rest(0)]
        else:
            hooks11[1] = [lambda: repair_rest(0)]
        block_mtouter(1, 1, et11, hooks=hooks11)
        select_pre(1)
        select_fin(1)
        if repair:
            repair_matmuls(1)
        repair_rest(1)

    nc.compile()
    return nc


_NC_CACHE = {}


def _get_nc(repair=REPAIR, dbg=False):
    key = (repair, dbg)
    if key not in _NC_CACHE:
        _NC_CACHE[key] = build(repair, dbg)
    return _NC_CACHE[key]


def make_in_maps(hidden_state, encoder_outputs, attn_w, attn_b, v):
    hidden_state = np.asarray(hidden_state, dtype=np.float32)
    encoder_outputs = np.asarray(encoder_outputs, dtype=np.float32)
    attn_w = np.asarray(attn_w, dtype=np.float32)
    attn_b = np.asarray(attn_b, dtype=np.float32)
    v = np.asarray(v, dtype=np.float32)

    w_e_t = np.ascontiguousarray(attn_w[:, H:].T)          # [E, H]
    wht_t = np.ascontiguousarray(attn_w[:, :H].T).astype(np.float16)
    encb = encoder_outputs.transpose(1, 0, 2)              # [B, S, E]

    # fp8 operands (scaled into e4m3's sweet spot)
    wet8_t = np.asarray(w_e_t * SC_W, dtype=ml_dtypes.float8_e4m3)
    wet8_t = np.ascontiguousarray(
        wet8_t.reshape(KP, 2, PT, H).transpose(0, 2, 1, 3))   # [KP,128,2,H]
    wet16_t = np.ascontiguousarray(w_e_t.reshape(KT, PT, H)).astype(np.float16)

    bt_t = np.repeat(
        attn_b.reshape(MT, PT).T[:, :, None], BL, axis=2
    ).reshape(PT, MT * BL)
    vt_t = np.ascontiguousarray(v.reshape(MT, PT).T)
    prep_t = (np.arange(128)[None, :] % 16 ==
              np.arange(PT)[:, None]).astype(np.float32)
    cf_t = np.concatenate(
        [bt_t, vt_t, np.full((PT, 1), -40.0, np.float32),
         np.ones((PT, 16), np.float32), prep_t], axis=1,
    ).astype(np.float32)
    pcol_t = ((np.arange(PT) % 16) * W).astype(np.float32).reshape(PT, 1)
    vrep_t = np.ascontiguousarray(
        np.repeat(v.reshape(MT, PT, 1), PT, axis=2).transpose(1, 0, 2)
        .reshape(PT, MT * PT)).astype(np.float16)

    in_maps = []
    for i in range(NCORES):
        rows = slice(i * BL, (i + 1) * BL)
        enc_c = encb[rows]                                 # [BL, S, E]
        encT = enc_c.transpose(0, 2, 1)                    # [BL, E, S]
        enc8_t = np.asarray(encT * SC_E, dtype=ml_dtypes.float8_e4m3)
        enc8_t = np.ascontiguousarray(
            enc8_t.reshape(BL, KP, 2, PT, S).transpose(0, 1, 3, 2, 4))
        in_maps.append(
            {
                "enc8": enc8_t,
                "wet8": wet8_t,
                "enc16": np.ascontiguousarray(enc_c).astype(np.float16),
                "wet16": wet16_t,
                "wht": wht_t,
                "ht": np.ascontiguousarray(
                    hidden_state[rows].T.reshape(KT_H, PT, BL)
                    .transpose(1, 0, 2).reshape(PT, KT_H * BL)
                ).astype(np.float16),
                "cf": cf_t,
                "vrep": vrep_t,
                "pcol": pcol_t,
            }
        )
    return in_maps


def run(inputs, trace=False, compute_dtype=None, dbg=False, **spmd_kwargs):
    nc = _get_nc(dbg=dbg)
    in_maps = make_in_maps(**inputs)
    res = run_bass_kernel_spmd(
        nc, in_maps, core_ids=list(range(NCORES)), trace=trace, **spmd_kwargs
    )
    out = np.concatenate(
        [res.results[i]["out"].reshape(BL, S) for i in range(NCORES)], axis=0
    )
    return out.astype(np.float32), res


def kernel(**inputs):
    out, _ = run(inputs, trace=False)
    return out


# revision 35
# speedup vs baseline: 1.4084x; 1.1409x over previous
"""Trainium2 Bass kernel for nn_Attention (additive/Bahdanau-style attention).

Math (reference):
    enc [S,B,2H] -> [B,S,2H]
    energy  = tanh(h @ Wh^T + enc @ We^T + b)    # [B,S,H]
    logits  = energy . v                         # [B,S]
    out     = softmax(logits, axis=S)            # [B,S]

Sharding: data-parallel over batch. B=16 rows over 8 NeuronCores -> 2 rows
per core; attn weights replicated. No collectives needed.

Per-core design (feature-major so each softmax row sits on one partition and
the tanh bias is a per-partition scalar):
  - enc pre-transposed on host to [b, e, s] fp16; We^T pre-transposed fp16.
  - Main matmul e_projT[o, s] = We^T.T @ encT, K=2048 accumulated in PSUM.
    fp16 streams at 1 row/cycle with fast weight load; ~112us of PE work
    dominates the kernel, so the schedule keeps the PE queue dense:
      * blocks (b=0, st=0/1) run kt-outer across all 8 PSUM banks so the PE
        consumes (wet, enc) DMA pairs as they land in the DMA-bound prefix;
      * later blocks run mt-outer so ScalarE tanh overlaps the next group;
      * slow reductions (DVE v-dot chains) are emitted 1-2 blocks late so
        the in-order PE queue never stalls on them.
  - h_proj runs as M=2 fp16 matmuls during the prefix; the tiny [2,1024] ->
    [128,16] transpose goes through a DRAM round-trip on the gpsimd queue.
  - energy tanh is fused on ScalarE: tanh(psum + (Wh h + b)[o]) via the
    per-partition bias port.
  - v-dot: DVE per-partition scale+add, one rounding to f32r, then a single
    full-rate f32r ones-matmul per 512-chunk contracts the partition dim.
    The final block instead defers per-mt fp16 v-dot matmuls on the PE so
    the kernel tail is short.
  - softmax: exp(x - 40) with a constant shift (logits here are ~[-36, 37];
    fp32 exp is finite below 88, so no max pass is needed), ScalarE
    accum_out produces the denominator in the same pass.
  - ~2us of junk matmuls pre-warm the PE HAM clock gate during the prologue.
"""

from contextlib import ExitStack

import numpy as np

import concourse.bacc as bacc
import concourse.mybir as mybir
import concourse.tile as tile
from concourse.bass_utils import run_bass_kernel_spmd

H = 1024
B = 16
S = 1024
E = 2 * H
NCORES = 8
BL = B // NCORES        # 2 batch rows per core

PT = 128                # partition tile
NT = 512                # free-dim tile (one fp32 PSUM bank)
KT_E = E // PT          # 16 K-tiles in the main matmul
MT = H // PT            # 8 output-feature tiles
ST = S // NT            # 2 seq chunks
KT_H = H // PT          # 8 K-tiles for h_proj

F32 = mybir.dt.float32
F16 = mybir.dt.float16
AF = mybir.ActivationFunctionType

# main-matmul operand dtype: "f16" (1 cyc/row, fast weight load),
# "f32r" (1 cyc/row, ~2x the precision), "f32" (exact, 4 cyc/row)
COMPUTE_DTYPE = "f16"


def build(compute_dtype=COMPUTE_DTYPE):
    cdt = {"f32r": mybir.dt.float32r, "f32": F32, "f16": F16}[compute_dtype]
    nc = bacc.Bacc("TRN2", target_bir_lowering=False, debug=False)

    enc = nc.dram_tensor("enc", [BL, E, S], cdt, kind="ExternalInput").ap()
    wet = nc.dram_tensor("wet", [E, H], cdt, kind="ExternalInput").ap()
    wht = nc.dram_tensor("wht", [H, H], F16, kind="ExternalInput").ap()
    ht = nc.dram_tensor("ht", [PT, KT_H * BL], F16, kind="ExternalInput").ap()
    cf = nc.dram_tensor("cf", [PT, KT_H * BL + MT + 1], F32,
                        kind="ExternalInput").ap()
    ones = nc.dram_tensor("ones", [PT, 1], mybir.dt.float32r,
                          kind="ExternalInput").ap()
    vtc = nc.dram_tensor("vtc", [PT, MT], cdt, kind="ExternalInput").ap()
    out = nc.dram_tensor("out", [BL, S], F32, kind="ExternalOutput").ap()
    hp_dram = nc.dram_tensor("hp_scratch", [BL, H], F32).ap()

    with tile.TileContext(nc) as tc, ExitStack() as ctx:
        constp = ctx.enter_context(tc.tile_pool(name="constp", bufs=1))
        wetp = ctx.enter_context(tc.tile_pool(name="wetp", bufs=KT_E))
        whtp = ctx.enter_context(tc.tile_pool(name="whtp", bufs=1))
        encp = ctx.enter_context(tc.tile_pool(name="encp", bufs=2 * KT_E))
        hpp = ctx.enter_context(tc.tile_pool(name="hpp", bufs=1))
        engp = ctx.enter_context(tc.tile_pool(name="engp", bufs=4))
        accp = ctx.enter_context(tc.tile_pool(name="accp", bufs=3))
        attp = ctx.enter_context(tc.tile_pool(name="attp", bufs=1))
        smp = ctx.enter_context(tc.tile_pool(name="smp", bufs=1))
        # one shared PSUM pool: every tile takes one bank-sized slot, so
        # block 0 can hold all 8 accumulation groups at once
        psp = ctx.enter_context(tc.tile_pool(name="psp", bufs=8, space="PSUM"))

        # ---- constants (ht first: the very first matmul needs it) -------
        ht_sb = constp.tile([PT, KT_H * BL], F16)
        nc.sync.dma_start(ht_sb[:], ht[:])

        # HAM pre-warm: ~2us of junk matmuls while the DMA prologue streams.
        # The PE clock gate opens after ~3.4us of activity, so phase A and
        # early block-0 matmuls then run at 2.4GHz instead of 1.2GHz.
        junk_ps = psp.tile([1, 2], F32, tag="ps", name="junk_ps2")
        for _ in range(100):
            nc.tensor.matmul(
                junk_ps[:], ht_sb[:, 0:1], ht_sb[:, 0:2],
                start=True, stop=True, skip_group_check=True,
            )

        # ---- phase A: hpb[o-tile][o, b] = (Wh @ h + attn_b) -------------
        # 1) hp[b, o] via M=2 matmuls, kt-outer so the PE tracks the wht DMA
        php = [
            psp.tile([BL, NT], F32, tag="ps", name=f"php{oc}")
            for oc in range(H // NT)
        ]
        wht_sb = whtp.tile([PT, KT_H * H], F16, name="wht_sb")
        wht_v = wht_sb[:].rearrange("p (k o) -> p k o", k=KT_H)
        for kt in range(KT_H):
            nc.sync.dma_start(wht_v[:, kt, :], wht[kt * PT : (kt + 1) * PT, :])
        cf_sb = constp.tile([PT, KT_H * BL + MT + 1], F32)
        nc.sync.dma_start(cf_sb[:], cf[:])
        bt_sb = cf_sb[:, 0 : KT_H * BL]
        vt_sb = cf_sb[:, KT_H * BL : KT_H * BL + MT]
        nshift = cf_sb[0:1, KT_H * BL + MT : KT_H * BL + MT + 1]
        ones_sb = constp.tile([PT, 1], mybir.dt.float32r)
        nc.sync.dma_start(ones_sb[:], ones[:])
        vtc_sb = constp.tile([PT, MT], cdt)
        nc.sync.dma_start(vtc_sb[:], vtc[:])
        for kt in range(KT_H):
            for oc in range(H // NT):
                nc.tensor.matmul(
                    php[oc][:],
                    ht_sb[:, kt * BL : (kt + 1) * BL],
                    wht_v[:, kt, oc * NT : (oc + 1) * NT],
                    start=(kt == 0),
                    stop=(kt == KT_H - 1),
                )
        hp_sb = hpp.tile([BL, H], F32)
        for oc in range(H // NT):
            nc.scalar.copy(hp_sb[:, oc * NT : (oc + 1) * NT], php[oc][:])
        # 2) transpose [b, o] -> [o-tiled, b] via a DMA round-trip through
        # DRAM on the gpsimd queue: tiny, fully off the PE/PSUM/sync-queue
        # critical path (needed only when the first tanh runs, ~40us later)
        nc.gpsimd.dma_start(hp_dram[:], hp_sb[:])
        hpt_sb = hpp.tile([PT, KT_H * BL], F32, name="hpt_sb")
        for b in range(BL):
            nc.gpsimd.dma_start(
                hpt_sb[:].rearrange("p (m b) -> p m b", b=BL)[:, :, b],
                hp_dram[b].rearrange("(m p) -> p m", p=PT),
            )
        hpb_sb = hpp.tile([PT, KT_H * BL], F32, name="hpb_sb")
        nc.vector.tensor_add(hpb_sb[:], hpt_sb[:], bt_sb[:])

        # ---- phase B: main matmul + tanh + v-dot ------------------------
        # att lives on partition 0 only: compute-engine APs must start at a
        # quarter-partition boundary, so batch rows go side-by-side in the
        # free dim instead of on partitions 0/1.
        ex_tiles = {}
        sm_tiles = {}
        for b in range(BL):
            ex_tiles[b] = attp.tile([1, S], F32, name=f"ex{b}", tag=f"ex{b}")
            for st in range(ST):
                sm_tiles[(b, st)] = attp.tile(
                    [1, 1], F32, name=f"sm{b}{st}", tag=f"sm{b}{st}"
                )

        def exp_chunk(pa, b, st):
            # Exp straight from the PSUM chunk -- no staging copy; the
            # denominator falls out of the same pass via accum_out
            nc.scalar.activation(
                ex_tiles[b][0:1, st * NT : (st + 1) * NT],
                pa[:],
                AF.Exp,
                bias=nshift,
                accum_out=sm_tiles[(b, st)][:],
            )

        def load_enc_tiles(b, st):
            ts = []
            for kt in range(KT_E):
                t = encp.tile([PT, NT], cdt, name="enc_t")
                nc.sync.dma_start(
                    t[:],
                    enc[b, kt * PT : (kt + 1) * PT, st * NT : (st + 1) * NT],
                )
                ts.append(t)
            return ts

        def tanh_vdot(pe_psum, acc, b, mt):
            # energy = tanh(e_proj + hpb); weighted partition-sum deferred to
            # a single fp32 ones-matmul per block (exact, cheap on PE)
            en = engp.tile([PT, NT], F32, name="en", tag="en")
            nc.scalar.activation(
                en[:], pe_psum[:], AF.Tanh,
                bias=hpb_sb[:, mt * BL + b : mt * BL + b + 1]
            )
            if mt == 0:
                nc.vector.tensor_scalar_mul(acc[:], en[:], vt_sb[:, 0:1])
            else:
                tmp = engp.tile([PT, NT], F32, name="tmp", tag="vtmp")
                nc.vector.tensor_scalar_mul(tmp[:], en[:], vt_sb[:, mt : mt + 1])
                nc.vector.tensor_add(acc[:], acc[:], tmp[:])

        def vdot_reduce_store(acc, b, st):
            # single rounding to f32r, then a full-rate f32r ones-matmul for
            # the exact-ish partition sum (fp32 matmul would be 4 cyc/row)
            acc_r = accp.tile([PT, NT], mybir.dt.float32r, name="acc_r",
                              tag="acc_r", bufs=2)
            nc.vector.tensor_copy(acc_r[:], acc[:])
            pa = psp.tile([1, NT], F32, tag="ps", name="pa")
            nc.tensor.matmul(pa[:], ones_sb[:, 0:1], acc_r[:], start=True, stop=True)
            exp_chunk(pa, b, st)

        def softmax_row(b):
            smt = smp.tile([1, 1], F32, tag="smt", name="smt")
            nc.vector.tensor_add(
                smt[:], sm_tiles[(b, 0)][:], sm_tiles[(b, 1)][:]
            )
            rs = smp.tile([1, 1], F32, tag="rs", name="rs")
            nc.vector.reciprocal(rs[:], smt[:])
            res = smp.tile([1, S], F32, tag="res", name="res")
            nc.vector.tensor_scalar_mul(res[:], ex_tiles[b][:], rs[:])
            nc.sync.dma_start(out[b : b + 1, :], res[:])

        # blocks (0,0) and (0,1): kt-outer with per-kt DMA emission so the
        # PE consumes tiles right as they land during the DMA-bound prefix.
        # Block (0,0) also interleaves the resident wet tiles as "pairs".
        wet_tiles = [None] * KT_E

        def block_ktouter(b, st, with_wet=False):
            pes = [
                psp.tile([PT, NT], F32, tag="ps", name=f"pes_{b}{st}_{mt}")
                for mt in range(MT)
            ]
            for kt in range(KT_E):
                if with_wet:
                    wt = wetp.tile([PT, H], cdt, name="wet_t")
                    nc.sync.dma_start(wt[:], wet[kt * PT : (kt + 1) * PT, :])
                    wet_tiles[kt] = wt
                t = encp.tile([PT, NT], cdt, name="enc_t")
                nc.sync.dma_start(
                    t[:], enc[b, kt * PT : (kt + 1) * PT, st * NT : (st + 1) * NT]
                )
                for mt in range(MT):
                    nc.tensor.matmul(
                        pes[mt][:],
                        wet_tiles[kt][:, mt * PT : (mt + 1) * PT],
                        t[:],
                        start=(kt == 0),
                        stop=(kt == KT_E - 1),
                    )
            acc = accp.tile([PT, NT], F32, name="acc", tag="acc")
            for mt in range(MT):
                tanh_vdot(pes[mt], acc, b, mt)
            return acc

        def block_mtouter(b, st, etiles):
            acc = accp.tile([PT, NT], F32, name="acc", tag="acc")
            for mt in range(MT):
                pe = psp.tile([PT, NT], F32, tag="ps", name="pe")
                for kt in range(KT_E):
                    nc.tensor.matmul(
                        pe[:],
                        wet_tiles[kt][:, mt * PT : (mt + 1) * PT],
                        etiles[kt][:],
                        start=(kt == 0),
                        stop=(kt == KT_E - 1),
                    )
                tanh_vdot(pe, acc, b, mt)
            return acc

        def block_mtouter_pevdot(b, st, etiles, after_mt1=None):
            # v-dot as f32r PE matmuls, each deferred behind the NEXT mt
            # group's matmuls so the PE never waits on a tanh
            vt_r = vtc_sb[:]
            pa = psp.tile([1, NT], F32, tag="ps", name="pa_pe")
            ens = [None] * MT
            for mt in range(MT):
                pe = psp.tile([PT, NT], F32, tag="ps", name="pe")
                for kt in range(KT_E):
                    nc.tensor.matmul(
                        pe[:],
                        wet_tiles[kt][:, mt * PT : (mt + 1) * PT],
                        etiles[kt][:],
                        start=(kt == 0),
                        stop=(kt == KT_E - 1),
                    )
                if mt > 0:
                    nc.tensor.matmul(
                        pa[:], vt_r[:, mt - 1 : mt], ens[mt - 1][:],
                        start=(mt == 1), stop=False,
                    )
                if mt == 1 and after_mt1 is not None:
                    after_mt1()
                en = engp.tile([PT, NT], cdt, name="en_r", tag="en")
                nc.scalar.activation(
                    en[:], pe[:], AF.Tanh,
                    bias=hpb_sb[:, mt * BL + b : mt * BL + b + 1],
                )
                ens[mt] = en
            nc.tensor.matmul(
                pa[:], vt_r[:, MT - 1 : MT], ens[MT - 1][:],
                start=False, stop=True,
            )
            exp_chunk(pa, b, st)

        # the ones-matmuls are deferred behind later blocks' matmul streams
        # so the in-order PE queue never stalls on a DVE accumulation chain
        acc00 = block_ktouter(0, 0, with_wet=True)
        acc01 = block_ktouter(0, 1)
        et10 = load_enc_tiles(1, 0)
        acc10 = block_mtouter(1, 0, et10)
        # emit the last block's loads BEFORE softmax(0): the sync queue is
        # in-order, and row 0's output DMA must not dam the enc stream
        et11 = load_enc_tiles(1, 1)
        vdot_reduce_store(acc00, 0, 0)
        vdot_reduce_store(acc01, 0, 1)
        softmax_row(0)
        # chunk (1,0)'s partition-sum is emitted mid-(1,1) so only the
        # final block's own chain remains on the kernel tail
        block_mtouter_pevdot(
            1, 1, et11, after_mt1=lambda: vdot_reduce_store(acc10, 1, 0)
        )
        softmax_row(1)

    nc.compile()
    return nc


_NC_CACHE = {}


def _get_nc(compute_dtype=COMPUTE_DTYPE):
    if compute_dtype not in _NC_CACHE:
        _NC_CACHE[compute_dtype] = build(compute_dtype)
    return _NC_CACHE[compute_dtype]


def make_in_maps(hidden_state, encoder_outputs, attn_w, attn_b, v,
                 compute_dtype=COMPUTE_DTYPE):
    hidden_state = np.asarray(hidden_state, dtype=np.float32)
    encoder_outputs = np.asarray(encoder_outputs, dtype=np.float32)
    attn_w = np.asarray(attn_w, dtype=np.float32)
    attn_b = np.asarray(attn_b, dtype=np.float32)
    v = np.asarray(v, dtype=np.float32)

    np_cdt = {"f32r": np.float32, "f32": np.float32, "f16": np.float16}[
        compute_dtype
    ]
    wet_t = np.ascontiguousarray(attn_w[:, H:].T).astype(np_cdt)
    wht_t = np.ascontiguousarray(attn_w[:, :H].T).astype(np.float16)
    enc_t = np.ascontiguousarray(
        encoder_outputs.transpose(1, 2, 0).astype(np_cdt)
    )  # [16, 2048, 1024]
    bt_t = np.repeat(
        attn_b.reshape(MT, PT).T[:, :, None], BL, axis=2
    ).reshape(PT, MT * BL)  # [128, 16]: column m*BL+b = attn_b chunk m
    vt_t = np.ascontiguousarray(v.reshape(MT, PT).T)
    cf_t = np.ascontiguousarray(np.concatenate(
        [bt_t, vt_t, np.full((PT, 1), -40.0, np.float32)], axis=1
    ))


    in_maps = []
    for i in range(NCORES):
        rows = slice(i * BL, (i + 1) * BL)
        in_maps.append(
            {
                "enc": enc_t[rows],
                "wet": wet_t,
                "wht": wht_t,
                "ht": np.ascontiguousarray(
                    hidden_state[rows].T.reshape(KT_H, PT, BL)
                    .transpose(1, 0, 2).reshape(PT, KT_H * BL)
                ).astype(np.float16),
                "cf": cf_t,
                "ones": np.ones((PT, 1), dtype=np.float32),
                "vtc": vt_t.astype(np_cdt),
            }
        )
    return in_maps


def run(inputs, trace=False, compute_dtype=COMPUTE_DTYPE, **spmd_kwargs):
    nc = _get_nc(compute_dtype)
    in_maps = make_in_maps(**inputs, compute_dtype=compute_dtype)
    res = run_bass_kernel_spmd(
        nc, in_maps, core_ids=list(range(NCORES)), trace=trace, **spmd_kwargs
    )
    out = np.concatenate([res.results[i]["out"] for i in range(NCORES)], axis=0)
    return out.astype(np.float32), res


def kernel(**inputs):
    out, _ = run(inputs, trace=False)
    return out

